# revision 21
# baseline (speedup 1.0000x reference)
"""Trainium2 Bass kernel for nn_MiddleFusionModule.

out = LayerNorm(node + sigmoid(node@Wg1 + (t@Wg2+bg)[seg]) * t[seg]),
t = relu(text@W1+b1)@W2+b2, over 131072 nodes sharded across 8 cores.

Strategy (one SPMD program, 8 data-parallel cores):
 - segment_ids are sorted, so t[seg] is piecewise-constant. The host
   pads every segment to a multiple of CHUNK (512) nodes and lays the
   chunks out so each 512-node chunk maps to exactly ONE segment.
   A tiny [64, nch] chunk->segment one-hot then lets the device gather
   the per-chunk text vectors with 2 one-time matmuls; the per-chunk
   epilogue consumes them as per-partition scalar operands (ACT bias /
   DVE scalar) instead of per-chunk one-hot gather matmuls.
 - node_feat arrives TRANSPOSED (feature-major [256, N]) in bf16: the
   big gate matmul needs no on-chip transpose and input DMA halves.
 - Main loop per chunk: 4 bf16 z-matmuls (PE) -> sigmoid+u-bias (ACT)
   -> enh = gate*t + node fused on DVE -> 8 bf16 PE transposes to
   node-major PSUM -> bn_stats/aggr (DVE) -> rstd via DVE reciprocal +
   ACT Sqrt -> ACT affine -> paired 1MB output DMAs. GPSIMD does
   nothing in the loop (its semaphore ops cost ~3us each).
"""

import os
import sys

for _p in ("/opt/trn_rl_repo", "/root/.axon_site/_ro/trn_rl_repo"):
    if os.path.isdir(_p) and _p not in sys.path:
        sys.path.insert(0, _p)

from contextlib import ExitStack

import numpy as np
import ml_dtypes

import concourse.bacc as bacc
import concourse.mybir as mybir
import concourse.tile as tile
from concourse.bass_utils import run_bass_kernel_spmd
from concourse.masks import make_identity

F32 = mybir.dt.float32
F32R = mybir.dt.float32r
BF16 = mybir.dt.bfloat16
AF = mybir.ActivationFunctionType
ALU = mybir.AluOpType
N_CORES = 8
D = 256          # node dim
TD = 768         # text dim
HD = 1024        # hidden dim
B = 64           # batch (segments)
CHUNK = 512      # nodes per chunk (every chunk within one segment)
LN_EPS = 1e-3


def _build(nch: int, apply_gb: bool):
    """Build the single SPMD program for `nch` chunks per core."""
    npc = nch * CHUNK
    nc = bacc.Bacc("TRN2", target_bir_lowering=False, debug=False,
                   num_devices=N_CORES)

    nodeT = nc.dram_tensor("nodeT", [D, npc], BF16, kind="ExternalInput")
    ohc = nc.dram_tensor("ohc", [B, nch], F32, kind="ExternalInput")
    textT = nc.dram_tensor("textT", [TD, B], BF16, kind="ExternalInput")
    w1 = nc.dram_tensor("w1", [TD, HD], BF16, kind="ExternalInput")
    b1 = nc.dram_tensor("b1", [1, HD], BF16, kind="ExternalInput")
    w2 = nc.dram_tensor("w2", [HD, D], F32, kind="ExternalInput")
    b2 = nc.dram_tensor("b2", [1, D], F32, kind="ExternalInput")
    wg1 = nc.dram_tensor("wg1", [D, D], BF16, kind="ExternalInput")
    wg2 = nc.dram_tensor("wg2", [D, D], F32, kind="ExternalInput")
    bg = nc.dram_tensor("bg", [1, D], F32, kind="ExternalInput")
    gamma = nc.dram_tensor("gamma", [1, D], F32, kind="ExternalInput")
    beta = nc.dram_tensor("beta", [1, D], F32, kind="ExternalInput")
    onesd = nc.dram_tensor("onesd", [1, B], F32, kind="ExternalInput")
    out = nc.dram_tensor("out", [npc, D], F32, kind="ExternalOutput")

    with tile.TileContext(nc) as tc:
        with ExitStack() as ctx:
            consts = ctx.enter_context(tc.tile_pool(name="consts", bufs=1))

            # ---- constants / weights in SBUF ----
            wg1_sb = consts.tile([128, 2, D], BF16)
            nc.sync.dma_start(out=wg1_sb, in_=wg1.rearrange("(c k) n -> k c n", c=2))
            b1_sb = consts.tile([1, HD], BF16)
            nc.sync.dma_start(out=b1_sb, in_=b1[:, :])
            ones64b = consts.tile([1, B], BF16)
            b2_sb = consts.tile([1, D], F32R)
            nc.sync.dma_start(out=b2_sb, in_=b2.bitcast(F32R)[:, :])
            bg_sb = consts.tile([1, D], F32R)
            nc.sync.dma_start(out=bg_sb, in_=bg.bitcast(F32R)[:, :])
            ones64 = consts.tile([1, B], F32R)
            nc.sync.dma_start(out=ones64, in_=onesd.bitcast(F32R)[:, :])
            nc.vector.tensor_copy(out=ones64b, in_=ones64.bitcast(F32))
            ohc_sb = consts.tile([B, nch], F32R)
            nc.sync.dma_start(out=ohc_sb, in_=ohc.bitcast(F32R)[:, :])
            ident = consts.tile([128, 128], F32)
            make_identity(nc, ident)
            identb = consts.tile([128, 128], BF16)
            nc.vector.tensor_copy(out=identb, in_=ident)
            # per-chunk text vectors (feature-major), gathered once
            uc_sb = consts.tile([128, 2, nch], F32)
            tc_sb = consts.tile([128, 2, nch], F32)

            def R(ap):
                return ap.bitcast(F32R)

            # ---- text MLP (one-time, tiny) ----
            with ExitStack() as mctx:
                mp = mctx.enter_context(tc.tile_pool(name="mlp", bufs=1))
                mps = mctx.enter_context(
                    tc.tile_pool(name="mlp_ps", bufs=1, space="PSUM"))
                tx_sb = mp.tile([128, 6, B], BF16)
                nc.sync.dma_start(out=tx_sb, in_=textT.rearrange("(c k) m -> k c m", c=6))
                w1_sb = mp.tile([128, 6, HD], BF16)
                w1v = w1.rearrange("(c k) n -> k c n", c=6)
                for k in range(6):
                    nc.sync.dma_start(out=w1_sb[:, k, :], in_=w1v[:, k, :])
                w2_sb = mp.tile([128, 8, D], F32R)
                nc.sync.dma_start(out=w2_sb, in_=w2.bitcast(F32R).rearrange("(c k) n -> k c n", c=8))
                wg2_sb = mp.tile([128, 2, D], F32R)
                nc.sync.dma_start(out=wg2_sb, in_=wg2.bitcast(F32R).rearrange("(c k) n -> k c n", c=2))
                ps_t1 = mps.tile([B, 2, 512], F32)
                for h in range(2):
                    for k in range(6):
                        nc.tensor.matmul(
                            ps_t1[:, h, :], tx_sb[:, k, :],
                            w1_sb[:, k, h * 512:(h + 1) * 512],
                            start=(k == 0), stop=False)
                    nc.tensor.matmul(
                        ps_t1[:, h, :], ones64b,
                        b1_sb[:, h * 512:(h + 1) * 512],
                        start=False, stop=True)
                t1_sb = mp.tile([B, 2, 512], F32)
                for h in range(2):
                    nc.scalar.activation(out=t1_sb[:, h, :], in_=ps_t1[:, h, :],
                                         func=AF.Relu)
                # transpose t1 -> t1T [1024, 64] as [128, 8, 64]
                t1T_sb = mp.tile([128, 8, B], F32R)
                ps_tr = mps.tile([128, B], F32)
                for j in range(8):
                    src = t1_sb[:, j // 4, (j % 4) * 128:(j % 4 + 1) * 128]
                    nc.tensor.matmul(ps_tr, src, ident[:B, :B],
                                     is_transpose=True, start=True, stop=True)
                    nc.vector.tensor_copy(out=t1T_sb[:, j, :], in_=ps_tr)
                t_sb = mp.tile([B, D], F32R)     # t rows [64, 256]
                u_sb = mp.tile([B, D], F32R)     # (t @ Wg2 + bg) rows
                ps_t = mps.tile([B, D], F32)
                for j in range(8):
                    nc.tensor.matmul(ps_t, R(t1T_sb[:, j, :]), R(w2_sb[:, j, :]),
                                     start=(j == 0), stop=False)
                nc.tensor.matmul(ps_t, R(ones64), R(b2_sb), start=False, stop=True)
                nc.vector.tensor_copy(out=t_sb, in_=ps_t)
                # transpose t -> tT [256, 64] as [128, 2, 64]
                tT_sb = mp.tile([128, 2, B], F32R)
                for c in range(2):
                    nc.tensor.matmul(ps_tr,
                                     t_sb[:, c * 128:(c + 1) * 128].bitcast(F32),
                                     ident[:B, :B],
                                     is_transpose=True, start=True, stop=True)
                    nc.vector.tensor_copy(out=tT_sb[:, c, :], in_=ps_tr)
                ps_u = mps.tile([B, D], F32)
                for c in range(2):
                    nc.tensor.matmul(ps_u, R(tT_sb[:, c, :]), R(wg2_sb[:, c, :]),
                                     start=(c == 0), stop=False)
                nc.tensor.matmul(ps_u, R(ones64), R(bg_sb), start=False, stop=True)
                nc.vector.tensor_copy(out=u_sb, in_=ps_u)
                # gather per-chunk vectors: uc[f, ch] = u[seg(ch), f]
                ps_g = mps.tile([128, nch], F32)
                for c in range(2):
                    nc.tensor.matmul(ps_g, R(u_sb[:, c * 128:(c + 1) * 128]),
                                     R(ohc_sb), start=True, stop=True)
                    nc.vector.tensor_copy(out=uc_sb[:, c, :], in_=ps_g)
                    nc.tensor.matmul(ps_g, R(t_sb[:, c * 128:(c + 1) * 128]),
                                     R(ohc_sb), start=True, stop=True)
                    nc.vector.tensor_copy(out=tc_sb[:, c, :], in_=ps_g)

            # ---- main loop ----
            GRP = 3          # chunks per rstd/Newton batch; pe_ps holds
            #                  GRP + 1 bufs for the delayed group flush
            inp = ctx.enter_context(tc.tile_pool(name="inp", bufs=6))
            work = ctx.enter_context(tc.tile_pool(name="work", bufs=4))
            grp = ctx.enter_context(tc.tile_pool(name="grp", bufs=2))
            pz = ctx.enter_context(tc.tile_pool(name="pz", bufs=2, space="PSUM"))
            pe_ps = ctx.enter_context(
                tc.tile_pool(name="pe_ps", bufs=GRP + 1, space="PSUM"))

            nodeTv = nodeT.rearrange("(c k) n -> k c n", c=2)
            outv = out.rearrange("(ch j p) f -> ch p j f", p=128, j=4)
            outv2 = out.rearrange("(c2 j p) f -> c2 p j f", p=128, j=8)

            gb_sb = None
            if apply_gb:
                gb_sb = consts.tile([128, 2, D], F32)
                for name, src, slot in (("g", gamma, 0), ("b", beta, 1)):
                    import concourse.bass as bass
                    bcast = bass.AP(tensor=src.ap().tensor, offset=0,
                                    ap=[[0, 128], [1, D]])
                    nc.gpsimd.dma_start(out=gb_sb[:, slot, :], in_=bcast)

            dma_cache = {}

            def front_half(ch):
                """DMA-in + z matmuls + sigmoid + fused enh for chunk ch."""
                if ch % 2 == 0:
                    n2 = inp.tile([128, 2, 2 * CHUNK], BF16, tag="node2")
                    hi = min((ch + 2) * CHUNK, npc)
                    nc.sync.dma_start(out=n2[:, :, :hi - ch * CHUNK],
                                      in_=nodeTv[:, :, ch * CHUNK:hi])
                    dma_cache["node"] = n2
                node_sb = dma_cache["node"][:, :, (ch % 2) * CHUNK:
                                            (ch % 2 + 1) * CHUNK]

                ps_z = pz.tile([128, 2, CHUNK], F32, tag="ps_z")
                for c in range(2):
                    for k in range(2):
                        nc.tensor.matmul(
                            ps_z[:, c, :],
                            wg1_sb[:, k, c * 128:(c + 1) * 128],
                            node_sb[:, k, :],
                            start=(k == 0), stop=(k == 1))

                gate_sb = work.tile([128, 2, CHUNK], BF16, tag="gate")
                enh_sb = work.tile([128, 2, CHUNK], BF16, tag="enh")
                for c in range(2):
                    nc.scalar.activation(out=gate_sb[:, c, :],
                                         in_=ps_z[:, c, :], func=AF.Sigmoid,
                                         bias=uc_sb[:, c, ch:ch + 1])
                    nc.vector.scalar_tensor_tensor(
                        out=enh_sb[:, c, :], in0=gate_sb[:, c, :],
                        scalar=tc_sb[:, c, ch:ch + 1],
                        in1=node_sb[:, c, :],
                        op0=ALU.mult, op1=ALU.add)
                return enh_sb

            live_ps = {}     # ch -> ps_e tile (until its affine)
            live_mv = {}     # group -> mv tile

            def back_stats(ch, enh_sb):
                """Transpose + LN stats for chunk ch (group slot ch%GRP)."""
                gi, s = divmod(ch, GRP)
                if s == 0:
                    live_mv[gi] = grp.tile([128, GRP, 2, 2, 2], F32,
                                           tag="mv", name="mv_grp")
                ps_e = pe_ps.tile([128, 2, 2, 256], BF16, tag="ps_e")
                live_ps[ch] = ps_e
                for j in range(4):
                    for c in range(2):
                        nc.tensor.matmul(
                            ps_e[:, j // 2, j % 2, c * 128:(c + 1) * 128],
                            enh_sb[:, c, j * 128:(j + 1) * 128],
                            identb, is_transpose=True,
                            start=True, stop=True, skip_group_check=True)

                st_sb = work.tile([128, 2, 2, 6], F32, tag="st")
                mv_sb = live_mv[gi]
                for b in range(2):
                    for g in range(2):
                        nc.vector.bn_stats(
                            out=st_sb[:, b, g, :],
                            in_=ps_e[:, b, g, :])
                        nc.vector.bn_aggr(out=mv_sb[:, s, b, g, :],
                                          in_=st_sb[:, b, g:g + 1, :])

            def back_affine(gi, n):
                """rstd for group gi's n chunks in one batch (recip-seeded
                Newton on DVE - no ACT Sqrt, so the ACT table never leaves
                the sigmoid set), then affine + paired output DMAs."""
                ch0 = gi * GRP
                mv_sb = live_mv.pop(gi)
                ve = grp.tile([128, GRP, 2, 2, 1], F32, tag="ve")
                y = grp.tile([128, GRP, 2, 2, 1], F32, tag="y")
                tmp = grp.tile([128, GRP, 2, 2, 1], F32, tag="tmp")
                negms = grp.tile([128, GRP, 2, 2, 1], F32, tag="negms")
                nc.vector.tensor_scalar_add(
                    out=ve[:, :n], in0=mv_sb[:, :n, :, :, 1:2],
                    scalar1=LN_EPS)
                nc.vector.reciprocal(out=y[:, :n], in_=ve[:, :n])
                nc.vector.tensor_scalar(out=y[:, :n], in0=y[:, :n],
                                        scalar1=0.5, scalar2=0.5,
                                        op0=ALU.mult, op1=ALU.add)
                for _ in range(2):
                    nc.vector.tensor_mul(out=tmp[:, :n], in0=y[:, :n],
                                         in1=y[:, :n])
                    nc.vector.tensor_mul(out=tmp[:, :n], in0=tmp[:, :n],
                                         in1=ve[:, :n])
                    nc.vector.tensor_scalar(out=tmp[:, :n], in0=tmp[:, :n],
                                            scalar1=-0.5, scalar2=1.5,
                                            op0=ALU.mult, op1=ALU.add)
                    nc.vector.tensor_mul(out=y[:, :n], in0=y[:, :n],
                                         in1=tmp[:, :n])
                nc.vector.scalar_tensor_tensor(
                    out=negms[:, :n], in0=mv_sb[:, :n, :, :, 0:1],
                    scalar=-1.0, in1=y[:, :n], op0=ALU.mult, op1=ALU.mult)
                for i in range(n):
                    ch = ch0 + i
                    ps_e = live_ps.pop(ch)
                    if ch % 2 == 0:
                        out2_sb = work.tile([128, 8, D], F32, tag="out2")
                        dma_cache["out2"] = out2_sb
                    out_sb = dma_cache["out2"][:, (ch % 2) * 4:
                                               (ch % 2) * 4 + 4, :]
                    for b in range(2):
                        for g in range(2):
                            j = 2 * b + g
                            nc.scalar.activation(
                                out=out_sb[:, j, :],
                                in_=ps_e[:, b, g, :],
                                func=AF.Identity,
                                bias=negms[:, i, b, g, :],
                                scale=y[:, i, b, g, :])
                    if apply_gb:
                        for j in range(4):
                            nc.vector.tensor_mul(out=out_sb[:, j, :],
                                                 in0=out_sb[:, j, :],
                                                 in1=gb_sb[:, 0, :])
                            nc.vector.tensor_add(out=out_sb[:, j, :],
                                                 in0=out_sb[:, j, :],
                                                 in1=gb_sb[:, 1, :])
                    if ch % 2 == 1:
                        nc.scalar.dma_start(out=outv2[ch // 2],
                                            in_=dma_cache["out2"])
                    elif ch == nch - 1:
                        nc.scalar.dma_start(out=outv[ch],
                                            in_=dma_cache["out2"][:, 0:4, :])

            # one-chunk software pipeline: chunk i's front half is emitted
            # before chunk i-1's back half so PE/ACT/DVE streams always have
            # ready work ahead of the cross-engine dependency chain. Group
            # flushes are DELAYED until the next group's first chunk has its
            # stats emitted, so the ACT queue holds a sigmoid between the
            # Newton chain and the affines that wait on it (no ACT stall at
            # group boundaries). pe_ps bufs must be GRP + 1.
            prev_enh = None
            for ch in range(nch + 1):
                if ch < nch:
                    cur_enh = front_half(ch)
                else:
                    cur_enh = None
                if prev_enh is not None:
                    bch = ch - 1
                    back_stats(bch, prev_enh)
                    if bch % GRP == 0 and bch > 0:
                        back_affine(bch // GRP - 1, GRP)
                    if bch == nch - 1:
                        back_affine(bch // GRP, bch % GRP + 1)
                prev_enh = cur_enh

    nc.compile()
    return nc


_NC_CACHE = {}


def _plan(seg, total):
    """Pad each segment to a CHUNK multiple; lay chunks out over cores.

    Returns (nch, chunk_seg [8*nch], node_index [8*nch*CHUNK] int64 with -1
    for padding)."""
    seg = np.asarray(seg)
    counts = np.bincount(seg, minlength=B)[:B]
    chunks_per_seg = (counts + CHUNK - 1) // CHUNK
    total_chunks = int(chunks_per_seg.sum())
    nch = (total_chunks + N_CORES - 1) // N_CORES
    nch = max(nch, 2)
    grid_chunks = N_CORES * nch

    chunk_seg = np.zeros(grid_chunks, np.int64)
    node_index = np.full(grid_chunks * CHUNK, -1, np.int64)
    starts = np.concatenate([[0], np.cumsum(counts)])
    pos = 0
    for s in range(B):
        n = int(counts[s])
        if n == 0:
            continue
        k = int(chunks_per_seg[s])
        chunk_seg[pos:pos + k] = s
        idx = np.arange(starts[s], starts[s] + n)
        node_index[pos * CHUNK: pos * CHUNK + n] = idx
        pos += k
    # remaining chunks (pos..grid) stay segment 0, all padding
    return nch, chunk_seg, node_index


def _make_in_maps(node_feat, text_feat, seg, W1, b1, W2, b2, Wg, bg,
                  ln_gamma, ln_beta, nch, chunk_seg, node_index):
    npc = nch * CHUNK
    node_feat = np.asarray(node_feat, dtype=np.float32)
    # gather into padded layout (zeros in padding), then bf16-transpose
    padded = np.zeros((N_CORES * npc, D), np.float32)
    valid = node_index >= 0
    padded[valid] = node_feat[node_index[valid]]
    nodeT = np.ascontiguousarray(
        padded.T.astype(ml_dtypes.bfloat16))            # [256, 8*npc]
    ohc = (chunk_seg[None, :] == np.arange(B)[:, None]).astype(np.float32)

    textT = np.ascontiguousarray(
        np.asarray(text_feat, np.float32).T.astype(ml_dtypes.bfloat16))
    shared = {
        "textT": textT,
        "w1": np.ascontiguousarray(
            np.asarray(W1, np.float32).astype(ml_dtypes.bfloat16)),
        "b1": np.asarray(b1, np.float32).astype(
            ml_dtypes.bfloat16).reshape(1, HD),
        "w2": np.asarray(W2, np.float32),
        "b2": np.asarray(b2, np.float32).reshape(1, D),
        "wg1": np.ascontiguousarray(
            np.asarray(Wg, np.float32)[:D].astype(ml_dtypes.bfloat16)),
        "wg2": np.ascontiguousarray(np.asarray(Wg, np.float32)[D:]),
        "bg": np.asarray(bg, np.float32).reshape(1, D),
        "gamma": np.asarray(ln_gamma, np.float32).reshape(1, D),
        "beta": np.asarray(ln_beta, np.float32).reshape(1, D),
        "onesd": np.ones((1, B), np.float32),
    }
    in_maps = []
    for c in range(N_CORES):
        m = dict(shared)
        m["nodeT"] = np.ascontiguousarray(nodeT[:, c * npc:(c + 1) * npc])
        m["ohc"] = np.ascontiguousarray(
            ohc[:, c * nch:(c + 1) * nch])
        in_maps.append(m)
    return in_maps


def kernel(node_feat, text_feat, segment_ids, W1, b1, W2, b2, Wg, bg,
           ln_gamma, ln_beta):
    total, d = node_feat.shape
    seg = np.asarray(segment_ids)
    nch, chunk_seg, node_index = _plan(seg, total)

    apply_gb = not (np.all(np.asarray(ln_gamma) == 1.0)
                    and np.all(np.asarray(ln_beta) == 0.0))

    key = (nch, apply_gb)
    if key not in _NC_CACHE:
        _NC_CACHE[key] = _build(nch, apply_gb)
    nc = _NC_CACHE[key]

    in_maps = _make_in_maps(node_feat, text_feat, seg, W1, b1, W2, b2, Wg,
                            bg, ln_gamma, ln_beta, nch, chunk_seg, node_index)

    res = run_bass_kernel_spmd(nc, in_maps, core_ids=list(range(N_CORES)))
    out_pad = np.concatenate(
        [res.results[c]["out"] for c in range(N_CORES)], axis=0)
    valid = node_index >= 0
    out = np.empty((total, D), np.float32)
    out[node_index[valid]] = out_pad[valid]
    return out


def bench_device(inputs, iters=12):
    """Steady-state wall time per on-device execution (8 cores, inputs
    device-resident, donated outputs chained call-to-call). Includes PJRT
    dispatch overhead; see run_traced for the profiled HW time."""
    import time

    import jax
    from jax.experimental.shard_map import shard_map
    from jax.sharding import Mesh, PartitionSpec

    import concourse.bass2jax as b2j
    import concourse.mybir as mb

    seg = np.asarray(inputs["segment_ids"])
    total = np.asarray(inputs["node_feat"]).shape[0]
    nch, chunk_seg, node_index = _plan(seg, total)
    key = (nch, False)
    if key not in _NC_CACHE:
        _NC_CACHE[key] = _build(nch, False)
    nc = _NC_CACHE[key]
    in_maps = _make_in_maps(
        inputs["node_feat"], inputs["text_feat"], seg, inputs["W1"],
        inputs["b1"], inputs["W2"], inputs["b2"], inputs["Wg"], inputs["bg"],
        inputs["ln_gamma"], inputs["ln_beta"], nch, chunk_seg, node_index)

    b2j.install_neuronx_cc_hook()
    partition_name = (nc.partition_id_tensor.name
                      if nc.partition_id_tensor else None)
    in_names, out_names, out_avals, zero_outs = [], [], [], []
    for alloc in nc.m.functions[0].allocations:
        if not isinstance(alloc, mb.MemoryLocationSet):
            continue
        name = alloc.memorylocations[0].name
        if alloc.kind == "ExternalInput":
            if name != partition_name:
                in_names.append(name)
        elif alloc.kind == "ExternalOutput":
            out_names.append(name)
            shape = tuple(alloc.tensor_shape)
            dtype = mb.dt.np(alloc.dtype)
            out_avals.append(jax.core.ShapedArray(shape, dtype))
            zero_outs.append(np.zeros(shape, dtype))
    n_params = len(in_names)
    n_outs = len(out_avals)
    in_names_all = list(in_names) + out_names
    if partition_name is not None:
        in_names_all.append(partition_name)
    donate = tuple(range(n_params, n_params + n_outs))

    def _body(*args):
        operands = list(args)
        if partition_name is not None:
            operands.append(b2j.partition_id_tensor())
        outs = b2j._bass_exec_p.bind(
            *operands, out_avals=tuple(out_avals), in_names=tuple(in_names_all),
            out_names=tuple(out_names), lowering_input_output_aliases=(),
            sim_require_finite=True, sim_require_nnan=True, nc=nc)
        return tuple(outs)

    devices = jax.devices()[:N_CORES]
    mesh = Mesh(np.asarray(devices), ("core",))
    sharded = jax.jit(
        shard_map(_body, mesh=mesh,
                  in_specs=(PartitionSpec("core"),) * (n_params + n_outs),
                  out_specs=(PartitionSpec("core"),) * n_outs,
                  check_rep=False),
        donate_argnums=donate, keep_unused=True)
    concat_in = [
        np.concatenate([np.asarray(in_maps[c][nm]) for c in range(N_CORES)],
                       axis=0)
        for nm in in_names]
    sh = jax.sharding.NamedSharding(mesh, PartitionSpec("core"))
    in_dev = [jax.device_put(a, sh) for a in concat_in]
    zs = [jax.device_put(
        np.zeros((N_CORES * z.shape[0], *z.shape[1:]), z.dtype), sh)
        for z in zero_outs]
    jax.block_until_ready(in_dev)
    jax.block_until_ready(zs)
    outs = sharded(*in_dev, *zs)
    jax.block_until_ready(outs)          # warm-up / compile
    times = []
    for it in range(iters):
        t0 = time.perf_counter()
        nxt = sharded(*in_dev, *outs)
        jax.block_until_ready(nxt)
        times.append(time.perf_counter() - t0)
        outs = nxt
    times.sort()
    return times[len(times) // 2], times


def run_traced(inputs, trace_cores=None):
    """Re-run with NTFF tracing; returns max-core exec time in ns (or None)."""
    global _LAST_TRACE
    seg = np.asarray(inputs["segment_ids"])
    total = np.asarray(inputs["node_feat"]).shape[0]
    nch, chunk_seg, node_index = _plan(seg, total)
    apply_gb = not (np.all(np.asarray(inputs["ln_gamma"]) == 1.0)
                    and np.all(np.asarray(inputs["ln_beta"]) == 0.0))
    key = (nch, apply_gb)
    if key not in _NC_CACHE:
        _NC_CACHE[key] = _build(nch, apply_gb)
    nc = _NC_CACHE[key]
    in_maps = _make_in_maps(
        inputs["node_feat"], inputs["text_feat"], seg, inputs["W1"],
        inputs["b1"], inputs["W2"], inputs["b2"], inputs["Wg"], inputs["bg"],
        inputs["ln_gamma"], inputs["ln_beta"], nch, chunk_seg, node_index)
    res = run_bass_kernel_spmd(nc, in_maps, core_ids=list(range(N_CORES)),
                               trace=True, trace_cores=trace_cores)
    _LAST_TRACE = res
    return res.exec_time_ns


# revision 25
# speedup vs baseline: 1.1055x; 1.1055x over previous
"""Trainium2 Bass kernel for nn_MiddleFusionModule.

out = LayerNorm(node + sigmoid(node@Wg1 + (t@Wg2+bg)[seg]) * t[seg]),
t = relu(text@W1+b1)@W2+b2, over 131072 nodes sharded across 8 cores.

Strategy (one SPMD program, 8 data-parallel cores):
 - segment_ids are sorted, so t[seg] is piecewise-constant. The host
   pads every segment to a multiple of CHUNK (512) nodes and lays the
   chunks out so each 512-node chunk maps to exactly ONE segment.
   A tiny [64, nch] chunk->segment one-hot then lets the device gather
   the per-chunk text vectors with 2 one-time matmuls; the per-chunk
   epilogue consumes them as per-partition scalar operands (ACT bias /
   DVE scalar) instead of per-chunk one-hot gather matmuls.
 - node_feat arrives TRANSPOSED (feature-major [256, N]) in bf16: the
   big gate matmul needs no on-chip transpose and input DMA halves.
 - Main loop per chunk: 4 bf16 z-matmuls (PE) -> sigmoid+u-bias (ACT)
   -> enh = gate*t + node fused on DVE -> 8 bf16 PE transposes to
   node-major PSUM -> bn_stats/aggr (DVE) -> rstd via DVE reciprocal +
   ACT Sqrt -> ACT affine -> paired 1MB output DMAs. GPSIMD does
   nothing in the loop (its semaphore ops cost ~3us each).
"""

import os
import sys

for _p in ("/opt/trn_rl_repo", "/root/.axon_site/_ro/trn_rl_repo"):
    if os.path.isdir(_p) and _p not in sys.path:
        sys.path.insert(0, _p)

from contextlib import ExitStack

import numpy as np
import ml_dtypes

import concourse.bacc as bacc
import concourse.mybir as mybir
import concourse.tile as tile
from concourse.bass_utils import run_bass_kernel_spmd
from concourse.masks import make_identity

F32 = mybir.dt.float32
F32R = mybir.dt.float32r
BF16 = mybir.dt.bfloat16
AF = mybir.ActivationFunctionType
ALU = mybir.AluOpType
N_CORES = 8
D = 256          # node dim
TD = 768         # text dim
HD = 1024        # hidden dim
B = 64           # batch (segments)
CHUNK = 512      # nodes per chunk (every chunk within one segment)
LN_EPS = 1e-3


def _build(nch: int, apply_gb: bool):
    """Build the single SPMD program for `nch` chunks per core."""
    npc = nch * CHUNK
    nc = bacc.Bacc("TRN2", target_bir_lowering=False, debug=False,
                   num_devices=N_CORES)

    nodeT = nc.dram_tensor("nodeT", [D, npc], BF16, kind="ExternalInput")
    ohc = nc.dram_tensor("ohc", [B, nch], F32, kind="ExternalInput")
    textT = nc.dram_tensor("textT", [TD, B], BF16, kind="ExternalInput")
    w1 = nc.dram_tensor("w1", [TD, HD], BF16, kind="ExternalInput")
    b1 = nc.dram_tensor("b1", [1, HD], BF16, kind="ExternalInput")
    w2 = nc.dram_tensor("w2", [HD, D], F32, kind="ExternalInput")
    b2 = nc.dram_tensor("b2", [1, D], F32, kind="ExternalInput")
    wg1 = nc.dram_tensor("wg1", [D, D], BF16, kind="ExternalInput")
    wg2 = nc.dram_tensor("wg2", [D, D], F32, kind="ExternalInput")
    bg = nc.dram_tensor("bg", [1, D], F32, kind="ExternalInput")
    gamma = nc.dram_tensor("gamma", [1, D], F32, kind="ExternalInput")
    beta = nc.dram_tensor("beta", [1, D], F32, kind="ExternalInput")
    onesd = nc.dram_tensor("onesd", [1, B], F32, kind="ExternalInput")
    out = nc.dram_tensor("out", [npc, D], F32, kind="ExternalOutput")

    with tile.TileContext(nc) as tc:
        with ExitStack() as ctx:
            consts = ctx.enter_context(tc.tile_pool(name="consts", bufs=1))

            # ---- constants / weights in SBUF ----
            wg1_sb = consts.tile([128, 2, D], BF16)
            nc.sync.dma_start(out=wg1_sb, in_=wg1.rearrange("(c k) n -> k c n", c=2))
            b1_sb = consts.tile([1, HD], BF16)
            nc.sync.dma_start(out=b1_sb, in_=b1[:, :])
            ones64b = consts.tile([1, B], BF16)
            b2_sb = consts.tile([1, D], F32R)
            nc.sync.dma_start(out=b2_sb, in_=b2.bitcast(F32R)[:, :])
            bg_sb = consts.tile([1, D], F32R)
            nc.sync.dma_start(out=bg_sb, in_=bg.bitcast(F32R)[:, :])
            ones64 = consts.tile([1, B], F32R)
            nc.sync.dma_start(out=ones64, in_=onesd.bitcast(F32R)[:, :])
            nc.vector.tensor_copy(out=ones64b, in_=ones64.bitcast(F32))
            ohc_sb = consts.tile([B, nch], F32R)
            nc.sync.dma_start(out=ohc_sb, in_=ohc.bitcast(F32R)[:, :])
            ident = consts.tile([128, 128], F32)
            make_identity(nc, ident)
            identb = consts.tile([128, 128], BF16)
            nc.vector.tensor_copy(out=identb, in_=ident)
            # per-chunk text vectors (feature-major), gathered once
            uc_sb = consts.tile([128, 2, nch], F32)
            tc_sb = consts.tile([128, 2, nch], F32)

            def R(ap):
                return ap.bitcast(F32R)

            # ---- text MLP (one-time, tiny) ----
            with ExitStack() as mctx:
                mp = mctx.enter_context(tc.tile_pool(name="mlp", bufs=1))
                mps = mctx.enter_context(
                    tc.tile_pool(name="mlp_ps", bufs=1, space="PSUM"))
                tx_sb = mp.tile([128, 6, B], BF16)
                nc.sync.dma_start(out=tx_sb, in_=textT.rearrange("(c k) m -> k c m", c=6))
                w1_sb = mp.tile([128, 6, HD], BF16)
                w1v = w1.rearrange("(c k) n -> k c n", c=6)
                for k in range(6):
                    nc.sync.dma_start(out=w1_sb[:, k, :], in_=w1v[:, k, :])
                w2_sb = mp.tile([128, 8, D], F32R)
                nc.sync.dma_start(out=w2_sb, in_=w2.bitcast(F32R).rearrange("(c k) n -> k c n", c=8))
                wg2_sb = mp.tile([128, 2, D], F32R)
                nc.sync.dma_start(out=wg2_sb, in_=wg2.bitcast(F32R).rearrange("(c k) n -> k c n", c=2))
                ps_t1 = mps.tile([B, 2, 512], F32)
                for h in range(2):
                    for k in range(6):
                        nc.tensor.matmul(
                            ps_t1[:, h, :], tx_sb[:, k, :],
                            w1_sb[:, k, h * 512:(h + 1) * 512],
                            start=(k == 0), stop=False)
                    nc.tensor.matmul(
                        ps_t1[:, h, :], ones64b,
                        b1_sb[:, h * 512:(h + 1) * 512],
                        start=False, stop=True)
                t1_sb = mp.tile([B, 2, 512], F32)
                for h in range(2):
                    nc.scalar.activation(out=t1_sb[:, h, :], in_=ps_t1[:, h, :],
                                         func=AF.Relu)
                # transpose t1 -> t1T [1024, 64] as [128, 8, 64]
                t1T_sb = mp.tile([128, 8, B], F32R)
                ps_tr = mps.tile([128, B], F32)
                for j in range(8):
                    src = t1_sb[:, j // 4, (j % 4) * 128:(j % 4 + 1) * 128]
                    nc.tensor.matmul(ps_tr, src, ident[:B, :B],
                                     is_transpose=True, start=True, stop=True)
                    nc.vector.tensor_copy(out=t1T_sb[:, j, :], in_=ps_tr)
                t_sb = mp.tile([B, D], F32R)     # t rows [64, 256]
                u_sb = mp.tile([B, D], F32R)     # (t @ Wg2 + bg) rows
                ps_t = mps.tile([B, D], F32)
                for j in range(8):
                    nc.tensor.matmul(ps_t, R(t1T_sb[:, j, :]), R(w2_sb[:, j, :]),
                                     start=(j == 0), stop=False)
                nc.tensor.matmul(ps_t, R(ones64), R(b2_sb), start=False, stop=True)
                nc.vector.tensor_copy(out=t_sb, in_=ps_t)
                # transpose t -> tT [256, 64] as [128, 2, 64]
                tT_sb = mp.tile([128, 2, B], F32R)
                for c in range(2):
                    nc.tensor.matmul(ps_tr,
                                     t_sb[:, c * 128:(c + 1) * 128].bitcast(F32),
                                     ident[:B, :B],
                                     is_transpose=True, start=True, stop=True)
                    nc.vector.tensor_copy(out=tT_sb[:, c, :], in_=ps_tr)
                ps_u = mps.tile([B, D], F32)
                for c in range(2):
                    nc.tensor.matmul(ps_u, R(tT_sb[:, c, :]), R(wg2_sb[:, c, :]),
                                     start=(c == 0), stop=False)
                nc.tensor.matmul(ps_u, R(ones64), R(bg_sb), start=False, stop=True)
                nc.vector.tensor_copy(out=u_sb, in_=ps_u)
                # gather per-chunk vectors: uc[f, ch] = u[seg(ch), f]
                ps_g = mps.tile([128, nch], F32)
                for c in range(2):
                    nc.tensor.matmul(ps_g, R(u_sb[:, c * 128:(c + 1) * 128]),
                                     R(ohc_sb), start=True, stop=True)
                    nc.vector.tensor_copy(out=uc_sb[:, c, :], in_=ps_g)
                    nc.tensor.matmul(ps_g, R(t_sb[:, c * 128:(c + 1) * 128]),
                                     R(ohc_sb), start=True, stop=True)
                    nc.vector.tensor_copy(out=tc_sb[:, c, :], in_=ps_g)

            # ---- main loop ----
            GRP = 4          # chunks per rstd/Newton batch; pe_ps holds
            #                  GRP + 1 bufs for the delayed group flush
            inp = ctx.enter_context(tc.tile_pool(name="inp", bufs=6))
            work = ctx.enter_context(tc.tile_pool(name="work", bufs=4))
            grp = ctx.enter_context(tc.tile_pool(name="grp", bufs=2))
            # single-bank ps_z tiles (one per feature half) leave 5 PSUM
            # banks for ps_e
            pz = ctx.enter_context(tc.tile_pool(name="pz", bufs=3, space="PSUM"))
            pe_ps = ctx.enter_context(
                tc.tile_pool(name="pe_ps", bufs=GRP + 1, space="PSUM"))

            nodeTv = nodeT.rearrange("(c k) n -> k c n", c=2)
            outv = out.rearrange("(ch j p) f -> ch p j f", p=128, j=4)
            outv2 = out.rearrange("(c2 j p) f -> c2 p j f", p=128, j=8)

            gb_sb = None
            if apply_gb:
                gb_sb = consts.tile([128, 2, D], F32)
                for name, src, slot in (("g", gamma, 0), ("b", beta, 1)):
                    import concourse.bass as bass
                    bcast = bass.AP(tensor=src.ap().tensor, offset=0,
                                    ap=[[0, 128], [1, D]])
                    nc.gpsimd.dma_start(out=gb_sb[:, slot, :], in_=bcast)

            dma_cache = {}

            def front_half(ch):
                """DMA-in + z matmuls + sigmoid + fused enh for chunk ch."""
                if ch % 2 == 0:
                    n2 = inp.tile([128, 2, 2 * CHUNK], BF16, tag="node2")
                    hi = min((ch + 2) * CHUNK, npc)
                    nc.sync.dma_start(out=n2[:, :, :hi - ch * CHUNK],
                                      in_=nodeTv[:, :, ch * CHUNK:hi])
                    dma_cache["node"] = n2
                node_sb = dma_cache["node"][:, :, (ch % 2) * CHUNK:
                                            (ch % 2 + 1) * CHUNK]

                gate_sb = work.tile([128, 2, CHUNK], BF16, tag="gate")
                enh_sb = work.tile([128, 2, CHUNK], BF16, tag="enh")
                for c in range(2):
                    ps_z = pz.tile([128, CHUNK], F32, tag="ps_z")
                    for k in range(2):
                        nc.tensor.matmul(
                            ps_z,
                            wg1_sb[:, k, c * 128:(c + 1) * 128],
                            node_sb[:, k, :],
                            start=(k == 0), stop=(k == 1))
                    nc.scalar.activation(out=gate_sb[:, c, :],
                                         in_=ps_z, func=AF.Sigmoid,
                                         bias=uc_sb[:, c, ch:ch + 1])
                    nc.vector.scalar_tensor_tensor(
                        out=enh_sb[:, c, :], in0=gate_sb[:, c, :],
                        scalar=tc_sb[:, c, ch:ch + 1],
                        in1=node_sb[:, c, :],
                        op0=ALU.mult, op1=ALU.add)
                return enh_sb

            live_ps = {}     # ch -> ps_e tile (until its affine)
            live_mv = {}     # group -> mv tile

            def back_stats(ch, enh_sb):
                """Transpose + LN stats for chunk ch (group slot ch%GRP)."""
                gi, s = divmod(ch, GRP)
                if s == 0:
                    live_mv[gi] = grp.tile([128, GRP, 2, 2, 2], F32,
                                           tag="mv", name="mv_grp")
                ps_e = pe_ps.tile([128, 2, 2, 256], BF16, tag="ps_e")
                live_ps[ch] = ps_e
                for j in range(4):
                    for c in range(2):
                        nc.tensor.matmul(
                            ps_e[:, j // 2, j % 2, c * 128:(c + 1) * 128],
                            enh_sb[:, c, j * 128:(j + 1) * 128],
                            identb, is_transpose=True,
                            start=True, stop=True, skip_group_check=True)

                st_sb = work.tile([128, 2, 2, 6], F32, tag="st")
                mv_sb = live_mv[gi]
                for b in range(2):
                    for g in range(2):
                        nc.vector.bn_stats(
                            out=st_sb[:, b, g, :],
                            in_=ps_e[:, b, g, :])
                        nc.vector.bn_aggr(out=mv_sb[:, s, b, g, :],
                                          in_=st_sb[:, b, g:g + 1, :])

            live_yn = {}     # group -> (y, negms)

            def group_newton(gi, n):
                """rstd for group gi's n chunks in one DVE batch (recip-seeded
                Newton - no ACT Sqrt, so the ACT table never leaves the
                sigmoid set). Emitted right after the group's last bn_aggr so
                it isn't queued behind the next chunk's stats."""
                mv_sb = live_mv.pop(gi)
                ve = grp.tile([128, GRP, 2, 2, 1], F32, tag="ve")
                y = grp.tile([128, GRP, 2, 2, 1], F32, tag="y")
                tmp = grp.tile([128, GRP, 2, 2, 1], F32, tag="tmp")
                negms = grp.tile([128, GRP, 2, 2, 1], F32, tag="negms")
                nc.vector.tensor_scalar_add(
                    out=ve[:, :n], in0=mv_sb[:, :n, :, :, 1:2],
                    scalar1=LN_EPS)
                nc.vector.reciprocal(out=y[:, :n], in_=ve[:, :n])
                nc.vector.tensor_scalar(out=y[:, :n], in0=y[:, :n],
                                        scalar1=0.5, scalar2=0.5,
                                        op0=ALU.mult, op1=ALU.add)
                for _ in range(2):
                    nc.vector.tensor_mul(out=tmp[:, :n], in0=y[:, :n],
                                         in1=y[:, :n])
                    nc.vector.tensor_mul(out=tmp[:, :n], in0=tmp[:, :n],
                                         in1=ve[:, :n])
                    nc.vector.tensor_scalar(out=tmp[:, :n], in0=tmp[:, :n],
                                            scalar1=-0.5, scalar2=1.5,
                                            op0=ALU.mult, op1=ALU.add)
                    nc.vector.tensor_mul(out=y[:, :n], in0=y[:, :n],
                                         in1=tmp[:, :n])
                nc.vector.scalar_tensor_tensor(
                    out=negms[:, :n], in0=mv_sb[:, :n, :, :, 0:1],
                    scalar=-1.0, in1=y[:, :n], op0=ALU.mult, op1=ALU.mult)
                live_yn[gi] = (y, negms)

            def group_affine(gi, n):
                """Affine + paired output DMAs for group gi (one iteration
                after its Newton, so ACT has sigmoid work as cover)."""
                ch0 = gi * GRP
                y, negms = live_yn.pop(gi)
                for i in range(n):
                    ch = ch0 + i
                    ps_e = live_ps.pop(ch)
                    if ch % 2 == 0:
                        out2_sb = work.tile([128, 8, D], F32, tag="out2")
                        dma_cache["out2"] = out2_sb
                    out_sb = dma_cache["out2"][:, (ch % 2) * 4:
                                               (ch % 2) * 4 + 4, :]
                    for b in range(2):
                        for g in range(2):
                            j = 2 * b + g
                            nc.scalar.activation(
                                out=out_sb[:, j, :],
                                in_=ps_e[:, b, g, :],
                                func=AF.Identity,
                                bias=negms[:, i, b, g, :],
                                scale=y[:, i, b, g, :])
                    if apply_gb:
                        for j in range(4):
                            nc.vector.tensor_mul(out=out_sb[:, j, :],
                                                 in0=out_sb[:, j, :],
                                                 in1=gb_sb[:, 0, :])
                            nc.vector.tensor_add(out=out_sb[:, j, :],
                                                 in0=out_sb[:, j, :],
                                                 in1=gb_sb[:, 1, :])
                    if ch % 2 == 1:
                        nc.scalar.dma_start(out=outv2[ch // 2],
                                            in_=dma_cache["out2"])
                    elif ch == nch - 1:
                        nc.scalar.dma_start(out=outv[ch],
                                            in_=dma_cache["out2"][:, 0:4, :])

            # one-chunk software pipeline: chunk i's front half is emitted
            # before chunk i-1's back half. The Newton batch for a group is
            # emitted right after its last bn_aggr (so DVE runs it before
            # the next chunk's stats), and the group's affines one iteration
            # later (so the ACT queue holds a sigmoid between the Newton
            # chain and the affines that wait on it). pe_ps bufs = GRP + 1.
            enh_live = {}
            for ch in range(nch + 2):
                if ch < nch:
                    enh_live[ch] = front_half(ch)
                bch = ch - 1
                if 0 <= bch < nch:
                    back_stats(bch, enh_live.pop(bch))
                    if bch % GRP == GRP - 1 or bch == nch - 1:
                        group_newton(bch // GRP, bch % GRP + 1)
                ach = ch - 2
                if 0 <= ach < nch and (ach % GRP == GRP - 1
                                       or ach == nch - 1):
                    group_affine(ach // GRP, ach % GRP + 1)

    nc.compile()
    return nc


_NC_CACHE = {}


def _plan(seg, total):
    """Pad each segment to a CHUNK multiple; lay chunks out over cores.

    Returns (nch, chunk_seg [8*nch], node_index [8*nch*CHUNK] int64 with -1
    for padding)."""
    seg = np.asarray(seg)
    counts = np.bincount(seg, minlength=B)[:B]
    chunks_per_seg = (counts + CHUNK - 1) // CHUNK
    total_chunks = int(chunks_per_seg.sum())
    nch = (total_chunks + N_CORES - 1) // N_CORES
    nch = max(nch, 2)
    grid_chunks = N_CORES * nch

    chunk_seg = np.zeros(grid_chunks, np.int64)
    node_index = np.full(grid_chunks * CHUNK, -1, np.int64)
    starts = np.concatenate([[0], np.cumsum(counts)])
    pos = 0
    for s in range(B):
        n = int(counts[s])
        if n == 0:
            continue
        k = int(chunks_per_seg[s])
        chunk_seg[pos:pos + k] = s
        idx = np.arange(starts[s], starts[s] + n)
        node_index[pos * CHUNK: pos * CHUNK + n] = idx
        pos += k
    # remaining chunks (pos..grid) stay segment 0, all padding
    return nch, chunk_seg, node_index


def _make_in_maps(node_feat, text_feat, seg, W1, b1, W2, b2, Wg, bg,
                  ln_gamma, ln_beta, nch, chunk_seg, node_index):
    npc = nch * CHUNK
    node_feat = np.asarray(node_feat, dtype=np.float32)
    # gather into padded layout (zeros in padding), then bf16-transpose
    padded = np.zeros((N_CORES * npc, D), np.float32)
    valid = node_index >= 0
    padded[valid] = node_feat[node_index[valid]]
    nodeT = np.ascontiguousarray(
        padded.T.astype(ml_dtypes.bfloat16))            # [256, 8*npc]
    ohc = (chunk_seg[None, :] == np.arange(B)[:, None]).astype(np.float32)

    textT = np.ascontiguousarray(
        np.asarray(text_feat, np.float32).T.astype(ml_dtypes.bfloat16))
    shared = {
        "textT": textT,
        "w1": np.ascontiguousarray(
            np.asarray(W1, np.float32).astype(ml_dtypes.bfloat16)),
        "b1": np.asarray(b1, np.float32).astype(
            ml_dtypes.bfloat16).reshape(1, HD),
        "w2": np.asarray(W2, np.float32),
        "b2": np.asarray(b2, np.float32).reshape(1, D),
        "wg1": np.ascontiguousarray(
            np.asarray(Wg, np.float32)[:D].astype(ml_dtypes.bfloat16)),
        "wg2": np.ascontiguousarray(np.asarray(Wg, np.float32)[D:]),
        "bg": np.asarray(bg, np.float32).reshape(1, D),
        "gamma": np.asarray(ln_gamma, np.float32).reshape(1, D),
        "beta": np.asarray(ln_beta, np.float32).reshape(1, D),
        "onesd": np.ones((1, B), np.float32),
    }
    in_maps = []
    for c in range(N_CORES):
        m = dict(shared)
        m["nodeT"] = np.ascontiguousarray(nodeT[:, c * npc:(c + 1) * npc])
        m["ohc"] = np.ascontiguousarray(
            ohc[:, c * nch:(c + 1) * nch])
        in_maps.append(m)
    return in_maps


def kernel(node_feat, text_feat, segment_ids, W1, b1, W2, b2, Wg, bg,
           ln_gamma, ln_beta):
    total, d = node_feat.shape
    seg = np.asarray(segment_ids)
    nch, chunk_seg, node_index = _plan(seg, total)

    apply_gb = not (np.all(np.asarray(ln_gamma) == 1.0)
                    and np.all(np.asarray(ln_beta) == 0.0))

    key = (nch, apply_gb)
    if key not in _NC_CACHE:
        _NC_CACHE[key] = _build(nch, apply_gb)
    nc = _NC_CACHE[key]

    in_maps = _make_in_maps(node_feat, text_feat, seg, W1, b1, W2, b2, Wg,
                            bg, ln_gamma, ln_beta, nch, chunk_seg, node_index)

    res = run_bass_kernel_spmd(nc, in_maps, core_ids=list(range(N_CORES)))
    out_pad = np.concatenate(
        [res.results[c]["out"] for c in range(N_CORES)], axis=0)
    valid = node_index >= 0
    out = np.empty((total, D), np.float32)
    out[node_index[valid]] = out_pad[valid]
    return out


def bench_device(inputs, iters=12):
    """Steady-state wall time per on-device execution (8 cores, inputs
    device-resident, donated outputs chained call-to-call). Includes PJRT
    dispatch overhead; see run_traced for the profiled HW time."""
    import time

    import jax
    from jax.experimental.shard_map import shard_map
    from jax.sharding import Mesh, PartitionSpec

    import concourse.bass2jax as b2j
    import concourse.mybir as mb

    seg = np.asarray(inputs["segment_ids"])
    total = np.asarray(inputs["node_feat"]).shape[0]
    nch, chunk_seg, node_index = _plan(seg, total)
    key = (nch, False)
    if key not in _NC_CACHE:
        _NC_CACHE[key] = _build(nch, False)
    nc = _NC_CACHE[key]
    in_maps = _make_in_maps(
        inputs["node_feat"], inputs["text_feat"], seg, inputs["W1"],
        inputs["b1"], inputs["W2"], inputs["b2"], inputs["Wg"], inputs["bg"],
        inputs["ln_gamma"], inputs["ln_beta"], nch, chunk_seg, node_index)

    b2j.install_neuronx_cc_hook()
    partition_name = (nc.partition_id_tensor.name
                      if nc.partition_id_tensor else None)
    in_names, out_names, out_avals, zero_outs = [], [], [], []
    for alloc in nc.m.functions[0].allocations:
        if not isinstance(alloc, mb.MemoryLocationSet):
            continue
        name = alloc.memorylocations[0].name
        if alloc.kind == "ExternalInput":
            if name != partition_name:
                in_names.append(name)
        elif alloc.kind == "ExternalOutput":
            out_names.append(name)
            shape = tuple(alloc.tensor_shape)
            dtype = mb.dt.np(alloc.dtype)
            out_avals.append(jax.core.ShapedArray(shape, dtype))
            zero_outs.append(np.zeros(shape, dtype))
    n_params = len(in_names)
    n_outs = len(out_avals)
    in_names_all = list(in_names) + out_names
    if partition_name is not None:
        in_names_all.append(partition_name)
    donate = tuple(range(n_params, n_params + n_outs))

    def _body(*args):
        operands = list(args)
        if partition_name is not None:
            operands.append(b2j.partition_id_tensor())
        outs = b2j._bass_exec_p.bind(
            *operands, out_avals=tuple(out_avals), in_names=tuple(in_names_all),
            out_names=tuple(out_names), lowering_input_output_aliases=(),
            sim_require_finite=True, sim_require_nnan=True, nc=nc)
        return tuple(outs)

    devices = jax.devices()[:N_CORES]
    mesh = Mesh(np.asarray(devices), ("core",))
    sharded = jax.jit(
        shard_map(_body, mesh=mesh,
                  in_specs=(PartitionSpec("core"),) * (n_params + n_outs),
                  out_specs=(PartitionSpec("core"),) * n_outs,
                  check_rep=False),
        donate_argnums=donate, keep_unused=True)
    concat_in = [
        np.concatenate([np.asarray(in_maps[c][nm]) for c in range(N_CORES)],
                       axis=0)
        for nm in in_names]
    sh = jax.sharding.NamedSharding(mesh, PartitionSpec("core"))
    in_dev = [jax.device_put(a, sh) for a in concat_in]
    zs = [jax.device_put(
        np.zeros((N_CORES * z.shape[0], *z.shape[1:]), z.dtype), sh)
        for z in zero_outs]
    jax.block_until_ready(in_dev)
    jax.block_until_ready(zs)
    outs = sharded(*in_dev, *zs)
    jax.block_until_ready(outs)          # warm-up / compile
    times = []
    for it in range(iters):
        t0 = time.perf_counter()
        nxt = sharded(*in_dev, *outs)
        jax.block_until_ready(nxt)
        times.append(time.perf_counter() - t0)
        outs = nxt
    times.sort()
    return times[len(times) // 2], times


def run_traced(inputs, trace_cores=None):
    """Re-run with NTFF tracing; returns max-core exec time in ns (or None)."""
    global _LAST_TRACE
    seg = np.asarray(inputs["segment_ids"])
    total = np.asarray(inputs["node_feat"]).shape[0]
    nch, chunk_seg, node_index = _plan(seg, total)
    apply_gb = not (np.all(np.asarray(inputs["ln_gamma"]) == 1.0)
                    and np.all(np.asarray(inputs["ln_beta"]) == 0.0))
    key = (nch, apply_gb)
    if key not in _NC_CACHE:
        _NC_CACHE[key] = _build(nch, apply_gb)
    nc = _NC_CACHE[key]
    in_maps = _make_in_maps(
        inputs["node_feat"], inputs["text_feat"], seg, inputs["W1"],
        inputs["b1"], inputs["W2"], inputs["b2"], inputs["Wg"], inputs["bg"],
        inputs["ln_gamma"], inputs["ln_beta"], nch, chunk_seg, node_index)
    res = run_bass_kernel_spmd(nc, in_maps, core_ids=list(range(N_CORES)),
                               trace=True, trace_cores=trace_cores)
    _LAST_TRACE = res
    return res.exec_time_ns


# revision 28
# speedup vs baseline: 1.1879x; 1.0746x over previous
"""Trainium2 Bass kernel for nn_MiddleFusionModule.

out = LayerNorm(node + sigmoid(node@Wg1 + (t@Wg2+bg)[seg]) * t[seg]),
t = relu(text@W1+b1)@W2+b2, over 131072 nodes sharded across 8 cores.

Strategy (one SPMD program, 8 data-parallel cores):
 - segment_ids are sorted, so t[seg] is piecewise-constant. The host
   pads every segment to a multiple of CHUNK (512) nodes and lays the
   chunks out so each 512-node chunk maps to exactly ONE segment.
   A tiny [64, nch] chunk->segment one-hot then lets the device gather
   the per-chunk text vectors with 2 one-time matmuls; the per-chunk
   epilogue consumes them as per-partition scalar operands (ACT bias /
   DVE scalar) instead of per-chunk one-hot gather matmuls.
 - node_feat arrives TRANSPOSED (feature-major [256, N]) in bf16: the
   big gate matmul needs no on-chip transpose and input DMA halves.
 - Main loop per chunk: 4 bf16 z-matmuls (PE) -> sigmoid+u-bias (ACT)
   -> enh = gate*t + node fused on DVE -> 8 bf16 PE transposes to
   node-major PSUM -> bn_stats/aggr (DVE) -> rstd via DVE reciprocal +
   ACT Sqrt -> ACT affine -> paired 1MB output DMAs. GPSIMD does
   nothing in the loop (its semaphore ops cost ~3us each).
"""

import os
import sys

for _p in ("/opt/trn_rl_repo", "/root/.axon_site/_ro/trn_rl_repo"):
    if os.path.isdir(_p) and _p not in sys.path:
        sys.path.insert(0, _p)

from contextlib import ExitStack

import numpy as np
import ml_dtypes

import concourse.bacc as bacc
import concourse.mybir as mybir
import concourse.tile as tile
from concourse.bass_utils import run_bass_kernel_spmd
from concourse.masks import make_identity

F32 = mybir.dt.float32
F32R = mybir.dt.float32r
BF16 = mybir.dt.bfloat16
AF = mybir.ActivationFunctionType
ALU = mybir.AluOpType
N_CORES = 8
D = 256          # node dim
TD = 768         # text dim
HD = 1024        # hidden dim
B = 64           # batch (segments)
CHUNK = 512      # nodes per chunk (every chunk within one segment)
LN_EPS = 1e-3


def _build(nch: int, apply_gb: bool):
    """Build the single SPMD program for `nch` chunks per core."""
    npc = nch * CHUNK
    nc = bacc.Bacc("TRN2", target_bir_lowering=False, debug=False,
                   num_devices=N_CORES)

    nodeT = nc.dram_tensor("nodeT", [D, npc], BF16, kind="ExternalInput")
    ohc = nc.dram_tensor("ohc", [B, nch], F32, kind="ExternalInput")
    textT = nc.dram_tensor("textT", [TD, B], BF16, kind="ExternalInput")
    w1 = nc.dram_tensor("w1", [TD, HD], BF16, kind="ExternalInput")
    b1 = nc.dram_tensor("b1", [1, HD], BF16, kind="ExternalInput")
    w2 = nc.dram_tensor("w2", [HD, D], F32, kind="ExternalInput")
    b2 = nc.dram_tensor("b2", [1, D], F32, kind="ExternalInput")
    wg1 = nc.dram_tensor("wg1", [D, D], BF16, kind="ExternalInput")
    wg2 = nc.dram_tensor("wg2", [D, D], F32, kind="ExternalInput")
    bg = nc.dram_tensor("bg", [1, D], F32, kind="ExternalInput")
    gamma = nc.dram_tensor("gamma", [1, D], F32, kind="ExternalInput")
    beta = nc.dram_tensor("beta", [1, D], F32, kind="ExternalInput")
    onesd = nc.dram_tensor("onesd", [1, B], F32, kind="ExternalInput")
    out = nc.dram_tensor("out", [npc, D], F32, kind="ExternalOutput")

    with tile.TileContext(nc) as tc:
        with ExitStack() as ctx:
            consts = ctx.enter_context(tc.tile_pool(name="consts", bufs=1))

            # ---- constants / weights in SBUF ----
            wg1_sb = consts.tile([128, 2, D], BF16)
            nc.sync.dma_start(out=wg1_sb, in_=wg1.rearrange("(c k) n -> k c n", c=2))
            b1_sb = consts.tile([1, HD], BF16)
            nc.sync.dma_start(out=b1_sb, in_=b1[:, :])
            ones64b = consts.tile([1, B], BF16)
            b2_sb = consts.tile([1, D], F32R)
            nc.sync.dma_start(out=b2_sb, in_=b2.bitcast(F32R)[:, :])
            bg_sb = consts.tile([1, D], F32R)
            nc.sync.dma_start(out=bg_sb, in_=bg.bitcast(F32R)[:, :])
            ones64 = consts.tile([1, B], F32R)
            nc.sync.dma_start(out=ones64, in_=onesd.bitcast(F32R)[:, :])
            nc.vector.tensor_copy(out=ones64b, in_=ones64.bitcast(F32))
            ohc_sb = consts.tile([B, nch], F32R)
            nc.sync.dma_start(out=ohc_sb, in_=ohc.bitcast(F32R)[:, :])
            ident = consts.tile([128, 128], F32)
            make_identity(nc, ident)
            identb = consts.tile([128, 128], BF16)
            nc.vector.tensor_copy(out=identb, in_=ident)
            # per-chunk text vectors (feature-major), gathered once
            uc_sb = consts.tile([128, 2, nch], F32)
            tc_sb = consts.tile([128, 2, nch], F32)

            def R(ap):
                return ap.bitcast(F32R)

            # ---- text MLP (one-time, tiny) ----
            with ExitStack() as mctx:
                mp = mctx.enter_context(tc.tile_pool(name="mlp", bufs=1))
                mps = mctx.enter_context(
                    tc.tile_pool(name="mlp_ps", bufs=1, space="PSUM"))
                tx_sb = mp.tile([128, 6, B], BF16)
                nc.sync.dma_start(out=tx_sb, in_=textT.rearrange("(c k) m -> k c m", c=6))
                w1_sb = mp.tile([128, 6, HD], BF16)
                w1v = w1.rearrange("(c k) n -> k c n", c=6)
                for k in range(6):
                    nc.sync.dma_start(out=w1_sb[:, k, :], in_=w1v[:, k, :])
                w2_sb = mp.tile([128, 8, D], F32R)
                nc.sync.dma_start(out=w2_sb, in_=w2.bitcast(F32R).rearrange("(c k) n -> k c n", c=8))
                wg2_sb = mp.tile([128, 2, D], F32R)
                nc.sync.dma_start(out=wg2_sb, in_=wg2.bitcast(F32R).rearrange("(c k) n -> k c n", c=2))
                ps_t1 = mps.tile([B, 2, 512], F32)
                for h in range(2):
                    for k in range(6):
                        nc.tensor.matmul(
                            ps_t1[:, h, :], tx_sb[:, k, :],
                            w1_sb[:, k, h * 512:(h + 1) * 512],
                            start=(k == 0), stop=False)
                    nc.tensor.matmul(
                        ps_t1[:, h, :], ones64b,
                        b1_sb[:, h * 512:(h + 1) * 512],
                        start=False, stop=True)
                t1_sb = mp.tile([B, 2, 512], F32)
                for h in range(2):
                    nc.scalar.activation(out=t1_sb[:, h, :], in_=ps_t1[:, h, :],
                                         func=AF.Relu)
                # transpose t1 -> t1T [1024, 64] as [128, 8, 64]
                t1T_sb = mp.tile([128, 8, B], F32R)
                ps_tr = mps.tile([128, B], F32)
                for j in range(8):
                    src = t1_sb[:, j // 4, (j % 4) * 128:(j % 4 + 1) * 128]
                    nc.tensor.matmul(ps_tr, src, ident[:B, :B],
                                     is_transpose=True, start=True, stop=True)
                    nc.vector.tensor_copy(out=t1T_sb[:, j, :], in_=ps_tr)
                t_sb = mp.tile([B, D], F32R)     # t rows [64, 256]
                u_sb = mp.tile([B, D], F32R)     # (t @ Wg2 + bg) rows
                ps_t = mps.tile([B, D], F32)
                for j in range(8):
                    nc.tensor.matmul(ps_t, R(t1T_sb[:, j, :]), R(w2_sb[:, j, :]),
                                     start=(j == 0), stop=False)
                nc.tensor.matmul(ps_t, R(ones64), R(b2_sb), start=False, stop=True)
                nc.vector.tensor_copy(out=t_sb, in_=ps_t)
                # transpose t -> tT [256, 64] as [128, 2, 64]
                tT_sb = mp.tile([128, 2, B], F32R)
                for c in range(2):
                    nc.tensor.matmul(ps_tr,
                                     t_sb[:, c * 128:(c + 1) * 128].bitcast(F32),
                                     ident[:B, :B],
                                     is_transpose=True, start=True, stop=True)
                    nc.vector.tensor_copy(out=tT_sb[:, c, :], in_=ps_tr)
                ps_u = mps.tile([B, D], F32)
                for c in range(2):
                    nc.tensor.matmul(ps_u, R(tT_sb[:, c, :]), R(wg2_sb[:, c, :]),
                                     start=(c == 0), stop=False)
                nc.tensor.matmul(ps_u, R(ones64), R(bg_sb), start=False, stop=True)
                nc.vector.tensor_copy(out=u_sb, in_=ps_u)
                # gather per-chunk vectors: uc[f, ch] = u[seg(ch), f]
                ps_g = mps.tile([128, nch], F32)
                for c in range(2):
                    nc.tensor.matmul(ps_g, R(u_sb[:, c * 128:(c + 1) * 128]),
                                     R(ohc_sb), start=True, stop=True)
                    nc.vector.tensor_copy(out=uc_sb[:, c, :], in_=ps_g)
                    nc.tensor.matmul(ps_g, R(t_sb[:, c * 128:(c + 1) * 128]),
                                     R(ohc_sb), start=True, stop=True)
                    nc.vector.tensor_copy(out=tc_sb[:, c, :], in_=ps_g)

            # ---- main loop ----
            GRP = 4          # chunks per rstd/Newton batch; pe_ps holds
            #                  GRP + 1 bufs for the delayed group flush
            inp = ctx.enter_context(tc.tile_pool(name="inp", bufs=6))
            work = ctx.enter_context(tc.tile_pool(name="work", bufs=4))
            grp = ctx.enter_context(tc.tile_pool(name="grp", bufs=2))
            # single-bank ps_z tiles (one per feature half) leave 6 PSUM
            # banks for ps_e
            pz = ctx.enter_context(tc.tile_pool(name="pz", bufs=2, space="PSUM"))
            pe_ps = ctx.enter_context(
                tc.tile_pool(name="pe_ps", bufs=GRP + 2, space="PSUM"))

            nodeTv = nodeT.rearrange("(c k) n -> k c n", c=2)
            outv = out.rearrange("(ch j p) f -> ch p j f", p=128, j=4)
            outv2 = out.rearrange("(c2 j p) f -> c2 p j f", p=128, j=8)

            gb_sb = None
            if apply_gb:
                gb_sb = consts.tile([128, 2, D], F32)
                for name, src, slot in (("g", gamma, 0), ("b", beta, 1)):
                    import concourse.bass as bass
                    bcast = bass.AP(tensor=src.ap().tensor, offset=0,
                                    ap=[[0, 128], [1, D]])
                    nc.gpsimd.dma_start(out=gb_sb[:, slot, :], in_=bcast)

            dma_cache = {}

            def front_half(ch):
                """DMA-in + z matmuls + sigmoid + fused enh for chunk ch."""
                if ch % 2 == 0:
                    n2 = inp.tile([128, 2, 2 * CHUNK], BF16, tag="node2")
                    hi = min((ch + 2) * CHUNK, npc)
                    nc.sync.dma_start(out=n2[:, :, :hi - ch * CHUNK],
                                      in_=nodeTv[:, :, ch * CHUNK:hi])
                    dma_cache["node"] = n2
                node_sb = dma_cache["node"][:, :, (ch % 2) * CHUNK:
                                            (ch % 2 + 1) * CHUNK]

                gate_sb = work.tile([128, 2, CHUNK], BF16, tag="gate")
                enh_sb = work.tile([128, 2, CHUNK], BF16, tag="enh")
                for c in range(2):
                    ps_z = pz.tile([128, CHUNK], F32, tag="ps_z")
                    for k in range(2):
                        nc.tensor.matmul(
                            ps_z,
                            wg1_sb[:, k, c * 128:(c + 1) * 128],
                            node_sb[:, k, :],
                            start=(k == 0), stop=(k == 1))
                    nc.scalar.activation(out=gate_sb[:, c, :],
                                         in_=ps_z, func=AF.Sigmoid,
                                         bias=uc_sb[:, c, ch:ch + 1])
                    nc.vector.scalar_tensor_tensor(
                        out=enh_sb[:, c, :], in0=gate_sb[:, c, :],
                        scalar=tc_sb[:, c, ch:ch + 1],
                        in1=node_sb[:, c, :],
                        op0=ALU.mult, op1=ALU.add)
                return enh_sb

            live_ps = {}     # ch -> ps_e tile (until its affine)
            live_mv = {}     # group -> mv tile

            def back_stats(ch, enh_sb):
                """Transpose + LN stats for chunk ch (group slot ch%GRP)."""
                gi, s = divmod(ch, GRP)
                if s == 0:
                    live_mv[gi] = grp.tile([128, GRP, 2, 2, 2], F32,
                                           tag="mv", name="mv_grp")
                ps_e = pe_ps.tile([128, 2, 2, 256], BF16, tag="ps_e")
                live_ps[ch] = ps_e
                for j in range(4):
                    for c in range(2):
                        nc.tensor.matmul(
                            ps_e[:, j // 2, j % 2, c * 128:(c + 1) * 128],
                            enh_sb[:, c, j * 128:(j + 1) * 128],
                            identb, is_transpose=True,
                            start=True, stop=True, skip_group_check=True)

                st_sb = work.tile([128, 2, 2, 6], F32, tag="st")
                mv_sb = live_mv[gi]
                for b in range(2):
                    for g in range(2):
                        nc.vector.bn_stats(
                            out=st_sb[:, b, g, :],
                            in_=ps_e[:, b, g, :])
                        nc.vector.bn_aggr(out=mv_sb[:, s, b, g, :],
                                          in_=st_sb[:, b, g:g + 1, :])

            live_yn = {}     # group -> (y, negms)

            def group_newton(gi, n):
                """rstd for group gi's n chunks in one DVE batch (recip-seeded
                Newton - no ACT Sqrt, so the ACT table never leaves the
                sigmoid set). Emitted right after the group's last bn_aggr so
                it isn't queued behind the next chunk's stats."""
                mv_sb = live_mv.pop(gi)
                ve = grp.tile([128, GRP, 2, 2, 1], F32, tag="ve")
                y = grp.tile([128, GRP, 2, 2, 1], F32, tag="y")
                tmp = grp.tile([128, GRP, 2, 2, 1], F32, tag="tmp")
                negms = grp.tile([128, GRP, 2, 2, 1], F32, tag="negms")
                nc.vector.tensor_scalar_add(
                    out=ve[:, :n], in0=mv_sb[:, :n, :, :, 1:2],
                    scalar1=LN_EPS)
                nc.vector.reciprocal(out=y[:, :n], in_=ve[:, :n])
                nc.vector.tensor_scalar(out=y[:, :n], in0=y[:, :n],
                                        scalar1=0.5, scalar2=0.5,
                                        op0=ALU.mult, op1=ALU.add)
                for _ in range(2):
                    nc.vector.tensor_mul(out=tmp[:, :n], in0=y[:, :n],
                                         in1=y[:, :n])
                    nc.vector.tensor_mul(out=tmp[:, :n], in0=tmp[:, :n],
                                         in1=ve[:, :n])
                    nc.vector.tensor_scalar(out=tmp[:, :n], in0=tmp[:, :n],
                                            scalar1=-0.5, scalar2=1.5,
                                            op0=ALU.mult, op1=ALU.add)
                    nc.vector.tensor_mul(out=y[:, :n], in0=y[:, :n],
                                         in1=tmp[:, :n])
                nc.vector.scalar_tensor_tensor(
                    out=negms[:, :n], in0=mv_sb[:, :n, :, :, 0:1],
                    scalar=-1.0, in1=y[:, :n], op0=ALU.mult, op1=ALU.mult)
                live_yn[gi] = (y, negms)

            def affine_chunk(ch):
                """Affine + paired output DMA for ONE chunk, emitted GRP+1
                iterations after its front so each loop iteration hands ACT
                a uniform 2-sigmoid + 4-affine mix (no burst/starve cycle)."""
                gi, i = divmod(ch, GRP)
                y, negms = live_yn[gi]
                ps_e = live_ps.pop(ch)
                if ch % 2 == 0:
                    out2_sb = work.tile([128, 8, D], F32, tag="out2")
                    dma_cache["out2"] = out2_sb
                out_sb = dma_cache["out2"][:, (ch % 2) * 4:
                                           (ch % 2) * 4 + 4, :]
                for b in range(2):
                    for g in range(2):
                        j = 2 * b + g
                        nc.scalar.activation(
                            out=out_sb[:, j, :],
                            in_=ps_e[:, b, g, :],
                            func=AF.Identity,
                            bias=negms[:, i, b, g, :],
                            scale=y[:, i, b, g, :])
                if apply_gb:
                    for j in range(4):
                        nc.vector.tensor_mul(out=out_sb[:, j, :],
                                             in0=out_sb[:, j, :],
                                             in1=gb_sb[:, 0, :])
                        nc.vector.tensor_add(out=out_sb[:, j, :],
                                             in0=out_sb[:, j, :],
                                             in1=gb_sb[:, 1, :])
                if ch % 2 == 1:
                    nc.scalar.dma_start(out=outv2[ch // 2],
                                        in_=dma_cache["out2"])
                elif ch == nch - 1:
                    nc.scalar.dma_start(out=outv[ch],
                                        in_=dma_cache["out2"][:, 0:4, :])
                if i == GRP - 1 or ch == nch - 1:
                    del live_yn[gi]

            # one-chunk software pipeline: chunk i's front half is emitted
            # before chunk i-1's back half. The Newton batch for a group is
            # emitted right after its last bn_aggr (so DVE runs it before
            # the next chunk's stats), and the group's affines one iteration
            # later (so the ACT queue holds a sigmoid between the Newton
            # chain and the affines that wait on it). pe_ps bufs = GRP + 1.
            enh_live = {}
            for ch in range(nch + GRP + 2):
                if ch < nch:
                    enh_live[ch] = front_half(ch)
                bch = ch - 1
                if 0 <= bch < nch:
                    back_stats(bch, enh_live.pop(bch))
                    if bch % GRP == GRP - 1 or bch == nch - 1:
                        group_newton(bch // GRP, bch % GRP + 1)
                ach = ch - (GRP + 1)
                if 0 <= ach < nch:
                    affine_chunk(ach)

    nc.compile()
    return nc


_NC_CACHE = {}


def _plan(seg, total):
    """Pad each segment to a CHUNK multiple; lay chunks out over cores.

    Returns (nch, chunk_seg [8*nch], node_index [8*nch*CHUNK] int64 with -1
    for padding)."""
    seg = np.asarray(seg)
    counts = np.bincount(seg, minlength=B)[:B]
    chunks_per_seg = (counts + CHUNK - 1) // CHUNK
    total_chunks = int(chunks_per_seg.sum())
    nch = (total_chunks + N_CORES - 1) // N_CORES
    nch = max(nch, 2)
    grid_chunks = N_CORES * nch

    chunk_seg = np.zeros(grid_chunks, np.int64)
    node_index = np.full(grid_chunks * CHUNK, -1, np.int64)
    starts = np.concatenate([[0], np.cumsum(counts)])
    pos = 0
    for s in range(B):
        n = int(counts[s])
        if n == 0:
            continue
        k = int(chunks_per_seg[s])
        chunk_seg[pos:pos + k] = s
        idx = np.arange(starts[s], starts[s] + n)
        node_index[pos * CHUNK: pos * CHUNK + n] = idx
        pos += k
    # remaining chunks (pos..grid) stay segment 0, all padding
    return nch, chunk_seg, node_index


def _make_in_maps(node_feat, text_feat, seg, W1, b1, W2, b2, Wg, bg,
                  ln_gamma, ln_beta, nch, chunk_seg, node_index):
    npc = nch * CHUNK
    node_feat = np.asarray(node_feat, dtype=np.float32)
    # gather into padded layout (zeros in padding), then bf16-transpose
    padded = np.zeros((N_CORES * npc, D), np.float32)
    valid = node_index >= 0
    padded[valid] = node_feat[node_index[valid]]
    nodeT = np.ascontiguousarray(
        padded.T.astype(ml_dtypes.bfloat16))            # [256, 8*npc]
    ohc = (chunk_seg[None, :] == np.arange(B)[:, None]).astype(np.float32)

    textT = np.ascontiguousarray(
        np.asarray(text_feat, np.float32).T.astype(ml_dtypes.bfloat16))
    shared = {
        "textT": textT,
        "w1": np.ascontiguousarray(
            np.asarray(W1, np.float32).astype(ml_dtypes.bfloat16)),
        "b1": np.asarray(b1, np.float32).astype(
            ml_dtypes.bfloat16).reshape(1, HD),
        "w2": np.asarray(W2, np.float32),
        "b2": np.asarray(b2, np.float32).reshape(1, D),
        "wg1": np.ascontiguousarray(
            np.asarray(Wg, np.float32)[:D].astype(ml_dtypes.bfloat16)),
        "wg2": np.ascontiguousarray(np.asarray(Wg, np.float32)[D:]),
        "bg": np.asarray(bg, np.float32).reshape(1, D),
        "gamma": np.asarray(ln_gamma, np.float32).reshape(1, D),
        "beta": np.asarray(ln_beta, np.float32).reshape(1, D),
        "onesd": np.ones((1, B), np.float32),
    }
    in_maps = []
    for c in range(N_CORES):
        m = dict(shared)
        m["nodeT"] = np.ascontiguousarray(nodeT[:, c * npc:(c + 1) * npc])
        m["ohc"] = np.ascontiguousarray(
            ohc[:, c * nch:(c + 1) * nch])
        in_maps.append(m)
    return in_maps


def kernel(node_feat, text_feat, segment_ids, W1, b1, W2, b2, Wg, bg,
           ln_gamma, ln_beta):
    total, d = node_feat.shape
    seg = np.asarray(segment_ids)
    nch, chunk_seg, node_index = _plan(seg, total)

    apply_gb = not (np.all(np.asarray(ln_gamma) == 1.0)
                    and np.all(np.asarray(ln_beta) == 0.0))

    key = (nch, apply_gb)
    if key not in _NC_CACHE:
        _NC_CACHE[key] = _build(nch, apply_gb)
    nc = _NC_CACHE[key]

    in_maps = _make_in_maps(node_feat, text_feat, seg, W1, b1, W2, b2, Wg,
                            bg, ln_gamma, ln_beta, nch, chunk_seg, node_index)

    res = run_bass_kernel_spmd(nc, in_maps, core_ids=list(range(N_CORES)))
    out_pad = np.concatenate(
        [res.results[c]["out"] for c in range(N_CORES)], axis=0)
    valid = node_index >= 0
    out = np.empty((total, D), np.float32)
    out[node_index[valid]] = out_pad[valid]
    return out


def bench_device(inputs, iters=12):
    """Steady-state wall time per on-device execution (8 cores, inputs
    device-resident, donated outputs chained call-to-call). Includes PJRT
    dispatch overhead; see run_traced for the profiled HW time."""
    import time

    import jax
    from jax.experimental.shard_map import shard_map
    from jax.sharding import Mesh, PartitionSpec

    import concourse.bass2jax as b2j
    import concourse.mybir as mb

    seg = np.asarray(inputs["segment_ids"])
    total = np.asarray(inputs["node_feat"]).shape[0]
    nch, chunk_seg, node_index = _plan(seg, total)
    key = (nch, False)
    if key not in _NC_CACHE:
        _NC_CACHE[key] = _build(nch, False)
    nc = _NC_CACHE[key]
    in_maps = _make_in_maps(
        inputs["node_feat"], inputs["text_feat"], seg, inputs["W1"],
        inputs["b1"], inputs["W2"], inputs["b2"], inputs["Wg"], inputs["bg"],
        inputs["ln_gamma"], inputs["ln_beta"], nch, chunk_seg, node_index)

    b2j.install_neuronx_cc_hook()
    partition_name = (nc.partition_id_tensor.name
                      if nc.partition_id_tensor else None)
    in_names, out_names, out_avals, zero_outs = [], [], [], []
    for alloc in nc.m.functions[0].allocations:
        if not isinstance(alloc, mb.MemoryLocationSet):
            continue
        name = alloc.memorylocations[0].name
        if alloc.kind == "ExternalInput":
            if name != partition_name:
                in_names.append(name)
        elif alloc.kind == "ExternalOutput":
            out_names.append(name)
            shape = tuple(alloc.tensor_shape)
            dtype = mb.dt.np(alloc.dtype)
            out_avals.append(jax.core.ShapedArray(shape, dtype))
            zero_outs.append(np.zeros(shape, dtype))
    n_params = len(in_names)
    n_outs = len(out_avals)
    in_names_all = list(in_names) + out_names
    if partition_name is not None:
        in_names_all.append(partition_name)
    donate = tuple(range(n_params, n_params + n_outs))

    def _body(*args):
        operands = list(args)
        if partition_name is not None:
            operands.append(b2j.partition_id_tensor())
        outs = b2j._bass_exec_p.bind(
            *operands, out_avals=tuple(out_avals), in_names=tuple(in_names_all),
            out_names=tuple(out_names), lowering_input_output_aliases=(),
            sim_require_finite=True, sim_require_nnan=True, nc=nc)
        return tuple(outs)

    devices = jax.devices()[:N_CORES]
    mesh = Mesh(np.asarray(devices), ("core",))
    sharded = jax.jit(
        shard_map(_body, mesh=mesh,
                  in_specs=(PartitionSpec("core"),) * (n_params + n_outs),
                  out_specs=(PartitionSpec("core"),) * n_outs,
                  check_rep=False),
        donate_argnums=donate, keep_unused=True)
    concat_in = [
        np.concatenate([np.asarray(in_maps[c][nm]) for c in range(N_CORES)],
                       axis=0)
        for nm in in_names]
    sh = jax.sharding.NamedSharding(mesh, PartitionSpec("core"))
    in_dev = [jax.device_put(a, sh) for a in concat_in]
    zs = [jax.device_put(
        np.zeros((N_CORES * z.shape[0], *z.shape[1:]), z.dtype), sh)
        for z in zero_outs]
    jax.block_until_ready(in_dev)
    jax.block_until_ready(zs)
    outs = sharded(*in_dev, *zs)
    jax.block_until_ready(outs)          # warm-up / compile
    times = []
    for it in range(iters):
        t0 = time.perf_counter()
        nxt = sharded(*in_dev, *outs)
        jax.block_until_ready(nxt)
        times.append(time.perf_counter() - t0)
        outs = nxt
    times.sort()
    return times[len(times) // 2], times


def run_traced(inputs, trace_cores=None):
    """Re-run with NTFF tracing; returns max-core exec time in ns (or None)."""
    global _LAST_TRACE
    seg = np.asarray(inputs["segment_ids"])
    total = np.asarray(inputs["node_feat"]).shape[0]
    nch, chunk_seg, node_index = _plan(seg, total)
    apply_gb = not (np.all(np.asarray(inputs["ln_gamma"]) == 1.0)
                    and np.all(np.asarray(inputs["ln_beta"]) == 0.0))
    key = (nch, apply_gb)
    if key not in _NC_CACHE:
        _NC_CACHE[key] = _build(nch, apply_gb)
    nc = _NC_CACHE[key]
    in_maps = _make_in_maps(
        inputs["node_feat"], inputs["text_feat"], seg, inputs["W1"],
        inputs["b1"], inputs["W2"], inputs["b2"], inputs["Wg"], inputs["bg"],
        inputs["ln_gamma"], inputs["ln_beta"], nch, chunk_seg, node_index)
    res = run_bass_kernel_spmd(nc, in_maps, core_ids=list(range(N_CORES)),
                               trace=True, trace_cores=trace_cores)
    _LAST_TRACE = res
    return res.exec_time_ns


# revision 31
# speedup vs baseline: 1.2067x; 1.0158x over previous
"""Trainium2 Bass kernel for nn_MiddleFusionModule.

out = LayerNorm(node + sigmoid(node@Wg1 + (t@Wg2+bg)[seg]) * t[seg]),
t = relu(text@W1+b1)@W2+b2, over 131072 nodes sharded across 8 cores.

Strategy (one SPMD program, 8 data-parallel cores):
 - segment_ids are sorted, so t[seg] is piecewise-constant. The host
   pads every segment to a multiple of CHUNK (512) nodes and lays the
   chunks out so each 512-node chunk maps to exactly ONE segment.
   A tiny [64, nch] chunk->segment one-hot then lets the device gather
   the per-chunk text vectors with 2 one-time matmuls; the per-chunk
   epilogue consumes them as per-partition scalar operands (ACT bias /
   DVE scalar) instead of per-chunk one-hot gather matmuls.
 - node_feat arrives TRANSPOSED (feature-major [256, N]) in bf16: the
   big gate matmul needs no on-chip transpose and input DMA halves.
 - Main loop per chunk: 4 bf16 z-matmuls (PE) -> sigmoid+u-bias (ACT)
   -> enh = gate*t + node fused on DVE -> 8 bf16 PE transposes to
   node-major PSUM -> bn_stats/aggr (DVE) -> rstd via DVE reciprocal +
   ACT Sqrt -> ACT affine -> paired 1MB output DMAs. GPSIMD does
   nothing in the loop (its semaphore ops cost ~3us each).
"""

import os
import sys

for _p in ("/opt/trn_rl_repo", "/root/.axon_site/_ro/trn_rl_repo"):
    if os.path.isdir(_p) and _p not in sys.path:
        sys.path.insert(0, _p)

from contextlib import ExitStack

import numpy as np
import ml_dtypes

import concourse.bacc as bacc
import concourse.mybir as mybir
import concourse.tile as tile
from concourse.bass_utils import run_bass_kernel_spmd
from concourse.masks import make_identity

F32 = mybir.dt.float32
F32R = mybir.dt.float32r
BF16 = mybir.dt.bfloat16
AF = mybir.ActivationFunctionType
ALU = mybir.AluOpType
N_CORES = 8
D = 256          # node dim
TD = 768         # text dim
HD = 1024        # hidden dim
B = 64           # batch (segments)
CHUNK = 512      # nodes per chunk (every chunk within one segment)
LN_EPS = 1e-3


def _build(nch: int, apply_gb: bool):
    """Build the single SPMD program for `nch` chunks per core."""
    npc = nch * CHUNK
    nc = bacc.Bacc("TRN2", target_bir_lowering=False, debug=False,
                   num_devices=N_CORES)

    nodeT = nc.dram_tensor("nodeT", [D, npc], BF16, kind="ExternalInput")
    ohc = nc.dram_tensor("ohc", [B, nch], F32, kind="ExternalInput")
    textT = nc.dram_tensor("textT", [TD, B], BF16, kind="ExternalInput")
    w1 = nc.dram_tensor("w1", [TD, HD], BF16, kind="ExternalInput")
    b1 = nc.dram_tensor("b1", [1, HD], BF16, kind="ExternalInput")
    w2 = nc.dram_tensor("w2", [HD, D], F32, kind="ExternalInput")
    b2 = nc.dram_tensor("b2", [1, D], F32, kind="ExternalInput")
    wg1 = nc.dram_tensor("wg1", [D, D], BF16, kind="ExternalInput")
    wg2 = nc.dram_tensor("wg2", [D, D], F32, kind="ExternalInput")
    bg = nc.dram_tensor("bg", [1, D], F32, kind="ExternalInput")
    gamma = nc.dram_tensor("gamma", [1, D], F32, kind="ExternalInput")
    beta = nc.dram_tensor("beta", [1, D], F32, kind="ExternalInput")
    onesd = nc.dram_tensor("onesd", [1, B], F32, kind="ExternalInput")
    out = nc.dram_tensor("out", [npc, D], F32, kind="ExternalOutput")

    with tile.TileContext(nc) as tc:
        with ExitStack() as ctx:
            consts = ctx.enter_context(tc.tile_pool(name="consts", bufs=1))

            # ---- constants / weights in SBUF ----
            wg1_sb = consts.tile([128, 2, D], BF16)
            nc.sync.dma_start(out=wg1_sb, in_=wg1.rearrange("(c k) n -> k c n", c=2))
            b1_sb = consts.tile([1, HD], BF16)
            nc.sync.dma_start(out=b1_sb, in_=b1[:, :])
            ones64b = consts.tile([1, B], BF16)
            b2_sb = consts.tile([1, D], F32R)
            nc.sync.dma_start(out=b2_sb, in_=b2.bitcast(F32R)[:, :])
            bg_sb = consts.tile([1, D], F32R)
            nc.sync.dma_start(out=bg_sb, in_=bg.bitcast(F32R)[:, :])
            ones64 = consts.tile([1, B], F32R)
            nc.sync.dma_start(out=ones64, in_=onesd.bitcast(F32R)[:, :])
            nc.vector.tensor_copy(out=ones64b, in_=ones64.bitcast(F32))
            ohc_sb = consts.tile([B, nch], F32R)
            nc.sync.dma_start(out=ohc_sb, in_=ohc.bitcast(F32R)[:, :])
            ident = consts.tile([128, 128], F32)
            make_identity(nc, ident)
            identb = consts.tile([128, 128], BF16)
            nc.vector.tensor_copy(out=identb, in_=ident)
            # per-chunk text vectors (feature-major), gathered once
            uc_sb = consts.tile([128, 2, nch], F32)
            tc_sb = consts.tile([128, 2, nch], F32)

            def R(ap):
                return ap.bitcast(F32R)

            # ---- text MLP (one-time, tiny) ----
            with ExitStack() as mctx:
                mp = mctx.enter_context(tc.tile_pool(name="mlp", bufs=1))
                mps = mctx.enter_context(
                    tc.tile_pool(name="mlp_ps", bufs=1, space="PSUM"))
                tx_sb = mp.tile([128, 6, B], BF16)
                nc.sync.dma_start(out=tx_sb, in_=textT.rearrange("(c k) m -> k c m", c=6))
                w1_sb = mp.tile([128, 6, HD], BF16)
                w1v = w1.rearrange("(c k) n -> k c n", c=6)
                for k in range(6):
                    nc.sync.dma_start(out=w1_sb[:, k, :], in_=w1v[:, k, :])
                w2_sb = mp.tile([128, 8, D], F32R)
                nc.sync.dma_start(out=w2_sb, in_=w2.bitcast(F32R).rearrange("(c k) n -> k c n", c=8))
                wg2_sb = mp.tile([128, 2, D], F32R)
                nc.sync.dma_start(out=wg2_sb, in_=wg2.bitcast(F32R).rearrange("(c k) n -> k c n", c=2))
                ps_t1 = mps.tile([B, 2, 512], F32)
                for h in range(2):
                    for k in range(6):
                        nc.tensor.matmul(
                            ps_t1[:, h, :], tx_sb[:, k, :],
                            w1_sb[:, k, h * 512:(h + 1) * 512],
                            start=(k == 0), stop=False)
                    nc.tensor.matmul(
                        ps_t1[:, h, :], ones64b,
                        b1_sb[:, h * 512:(h + 1) * 512],
                        start=False, stop=True)
                t1_sb = mp.tile([B, 2, 512], F32)
                for h in range(2):
                    nc.scalar.activation(out=t1_sb[:, h, :], in_=ps_t1[:, h, :],
                                         func=AF.Relu)
                # transpose t1 -> t1T [1024, 64] as [128, 8, 64]; distinct
                # PSUM slots per transpose so PE never waits on the copy
                t1T_sb = mp.tile([128, 8, B], F32R)
                ps_tr8 = mps.tile([128, 8, B], F32)
                for j in range(8):
                    src = t1_sb[:, j // 4, (j % 4) * 128:(j % 4 + 1) * 128]
                    nc.tensor.matmul(ps_tr8[:, j, :], src, ident[:B, :B],
                                     is_transpose=True, start=True, stop=True,
                                     skip_group_check=True)
                nc.vector.tensor_copy(out=t1T_sb, in_=ps_tr8)
                t_sb = mp.tile([B, D], F32R)     # t rows [64, 256]
                u_sb = mp.tile([B, D], F32R)     # (t @ Wg2 + bg) rows
                ps_t = mps.tile([B, D], F32)
                for j in range(8):
                    nc.tensor.matmul(ps_t, R(t1T_sb[:, j, :]), R(w2_sb[:, j, :]),
                                     start=(j == 0), stop=False)
                nc.tensor.matmul(ps_t, R(ones64), R(b2_sb), start=False, stop=True)
                nc.vector.tensor_copy(out=t_sb, in_=ps_t)
                # transpose t -> tT [256, 64] as [128, 2, 64]
                tT_sb = mp.tile([128, 2, B], F32R)
                ps_tr2 = mps.tile([128, 2, B], F32)
                for c in range(2):
                    nc.tensor.matmul(ps_tr2[:, c, :],
                                     t_sb[:, c * 128:(c + 1) * 128].bitcast(F32),
                                     ident[:B, :B],
                                     is_transpose=True, start=True, stop=True,
                                     skip_group_check=True)
                nc.vector.tensor_copy(out=tT_sb, in_=ps_tr2)
                ps_u = mps.tile([B, D], F32)
                for c in range(2):
                    nc.tensor.matmul(ps_u, R(tT_sb[:, c, :]), R(wg2_sb[:, c, :]),
                                     start=(c == 0), stop=False)
                nc.tensor.matmul(ps_u, R(ones64), R(bg_sb), start=False, stop=True)
                nc.vector.tensor_copy(out=u_sb, in_=ps_u)
                # gather per-chunk vectors: uc[f, ch] = u[seg(ch), f]
                ps_g = mps.tile([128, 4, nch], F32)
                for c in range(2):
                    nc.tensor.matmul(ps_g[:, c, :],
                                     R(u_sb[:, c * 128:(c + 1) * 128]),
                                     R(ohc_sb), start=True, stop=True,
                                     skip_group_check=True)
                    nc.tensor.matmul(ps_g[:, 2 + c, :],
                                     R(t_sb[:, c * 128:(c + 1) * 128]),
                                     R(ohc_sb), start=True, stop=True,
                                     skip_group_check=True)
                nc.vector.tensor_copy(out=uc_sb, in_=ps_g[:, 0:2, :])
                nc.vector.tensor_copy(out=tc_sb, in_=ps_g[:, 2:4, :])

            # ---- main loop ----
            GRP = 4          # chunks per rstd/Newton batch; pe_ps holds
            #                  GRP + 1 bufs for the delayed group flush
            inp = ctx.enter_context(tc.tile_pool(name="inp", bufs=6))
            work = ctx.enter_context(tc.tile_pool(name="work", bufs=4))
            grp = ctx.enter_context(tc.tile_pool(name="grp", bufs=2))
            # single-bank ps_z tiles (one per feature half) leave 6 PSUM
            # banks for ps_e
            pz = ctx.enter_context(tc.tile_pool(name="pz", bufs=2, space="PSUM"))
            pe_ps = ctx.enter_context(
                tc.tile_pool(name="pe_ps", bufs=GRP + 2, space="PSUM"))

            nodeTv = nodeT.rearrange("(c k) n -> k c n", c=2)
            outv = out.rearrange("(ch j p) f -> ch p j f", p=128, j=4)
            outv2 = out.rearrange("(c2 j p) f -> c2 p j f", p=128, j=8)

            gb_sb = None
            if apply_gb:
                gb_sb = consts.tile([128, 2, D], F32)
                for name, src, slot in (("g", gamma, 0), ("b", beta, 1)):
                    import concourse.bass as bass
                    bcast = bass.AP(tensor=src.ap().tensor, offset=0,
                                    ap=[[0, 128], [1, D]])
                    nc.gpsimd.dma_start(out=gb_sb[:, slot, :], in_=bcast)

            dma_cache = {}

            def front_half(ch):
                """DMA-in + z matmuls + sigmoid + fused enh for chunk ch."""
                if ch % 2 == 0:
                    n2 = inp.tile([128, 2, 2 * CHUNK], BF16, tag="node2")
                    hi = min((ch + 2) * CHUNK, npc)
                    nc.sync.dma_start(out=n2[:, :, :hi - ch * CHUNK],
                                      in_=nodeTv[:, :, ch * CHUNK:hi])
                    dma_cache["node"] = n2
                node_sb = dma_cache["node"][:, :, (ch % 2) * CHUNK:
                                            (ch % 2 + 1) * CHUNK]

                gate_sb = work.tile([128, 2, CHUNK], BF16, tag="gate")
                enh_sb = work.tile([128, 2, CHUNK], BF16, tag="enh")
                for c in range(2):
                    ps_z = pz.tile([128, CHUNK], F32, tag="ps_z")
                    for k in range(2):
                        nc.tensor.matmul(
                            ps_z,
                            wg1_sb[:, k, c * 128:(c + 1) * 128],
                            node_sb[:, k, :],
                            start=(k == 0), stop=(k == 1))
                    nc.scalar.activation(out=gate_sb[:, c, :],
                                         in_=ps_z, func=AF.Sigmoid,
                                         bias=uc_sb[:, c, ch:ch + 1])
                    nc.vector.scalar_tensor_tensor(
                        out=enh_sb[:, c, :], in0=gate_sb[:, c, :],
                        scalar=tc_sb[:, c, ch:ch + 1],
                        in1=node_sb[:, c, :],
                        op0=ALU.mult, op1=ALU.add)
                return enh_sb

            live_ps = {}     # ch -> ps_e tile (until its affine)
            live_mv = {}     # group -> mv tile

            def back_stats(ch, enh_sb):
                """Transpose + LN stats for chunk ch (group slot ch%GRP)."""
                gi, s = divmod(ch, GRP)
                if s == 0:
                    live_mv[gi] = grp.tile([128, GRP, 2, 2, 2], F32,
                                           tag="mv", name="mv_grp")
                ps_e = pe_ps.tile([128, 2, 2, 256], BF16, tag="ps_e")
                live_ps[ch] = ps_e
                for j in range(4):
                    for c in range(2):
                        nc.tensor.matmul(
                            ps_e[:, j // 2, j % 2, c * 128:(c + 1) * 128],
                            enh_sb[:, c, j * 128:(j + 1) * 128],
                            identb, is_transpose=True,
                            start=True, stop=True, skip_group_check=True)

                st_sb = work.tile([128, 2, 2, 6], F32, tag="st")
                mv_sb = live_mv[gi]
                for b in range(2):
                    for g in range(2):
                        nc.vector.bn_stats(
                            out=st_sb[:, b, g, :],
                            in_=ps_e[:, b, g, :])
                        nc.vector.bn_aggr(out=mv_sb[:, s, b, g, :],
                                          in_=st_sb[:, b, g:g + 1, :])

            live_yn = {}     # group -> (y, negms)

            def group_newton(gi, n):
                """rstd for group gi's n chunks in one DVE batch (recip-seeded
                Newton - no ACT Sqrt, so the ACT table never leaves the
                sigmoid set). Emitted right after the group's last bn_aggr so
                it isn't queued behind the next chunk's stats."""
                mv_sb = live_mv.pop(gi)
                ve = grp.tile([128, GRP, 2, 2, 1], F32, tag="ve")
                y = grp.tile([128, GRP, 2, 2, 1], F32, tag="y")
                tmp = grp.tile([128, GRP, 2, 2, 1], F32, tag="tmp")
                negms = grp.tile([128, GRP, 2, 2, 1], F32, tag="negms")
                nc.vector.tensor_scalar_add(
                    out=ve[:, :n], in0=mv_sb[:, :n, :, :, 1:2],
                    scalar1=LN_EPS)
                nc.vector.reciprocal(out=y[:, :n], in_=ve[:, :n])
                nc.vector.tensor_scalar(out=y[:, :n], in0=y[:, :n],
                                        scalar1=0.5, scalar2=0.5,
                                        op0=ALU.mult, op1=ALU.add)
                for _ in range(2):
                    nc.vector.tensor_mul(out=tmp[:, :n], in0=y[:, :n],
                                         in1=y[:, :n])
                    nc.vector.tensor_mul(out=tmp[:, :n], in0=tmp[:, :n],
                                         in1=ve[:, :n])
                    nc.vector.tensor_scalar(out=tmp[:, :n], in0=tmp[:, :n],
                                            scalar1=-0.5, scalar2=1.5,
                                            op0=ALU.mult, op1=ALU.add)
                    nc.vector.tensor_mul(out=y[:, :n], in0=y[:, :n],
                                         in1=tmp[:, :n])
                nc.vector.scalar_tensor_tensor(
                    out=negms[:, :n], in0=mv_sb[:, :n, :, :, 0:1],
                    scalar=-1.0, in1=y[:, :n], op0=ALU.mult, op1=ALU.mult)
                live_yn[gi] = (y, negms)

            def affine_chunk(ch):
                """Affine + paired output DMA for ONE chunk, emitted GRP+1
                iterations after its front so each loop iteration hands ACT
                a uniform 2-sigmoid + 4-affine mix (no burst/starve cycle)."""
                gi, i = divmod(ch, GRP)
                y, negms = live_yn[gi]
                ps_e = live_ps.pop(ch)
                if ch % 2 == 0:
                    out2_sb = work.tile([128, 8, D], F32, tag="out2")
                    dma_cache["out2"] = out2_sb
                out_sb = dma_cache["out2"][:, (ch % 2) * 4:
                                           (ch % 2) * 4 + 4, :]
                for b in range(2):
                    for g in range(2):
                        j = 2 * b + g
                        nc.scalar.activation(
                            out=out_sb[:, j, :],
                            in_=ps_e[:, b, g, :],
                            func=AF.Identity,
                            bias=negms[:, i, b, g, :],
                            scale=y[:, i, b, g, :])
                if apply_gb:
                    for j in range(4):
                        nc.vector.tensor_mul(out=out_sb[:, j, :],
                                             in0=out_sb[:, j, :],
                                             in1=gb_sb[:, 0, :])
                        nc.vector.tensor_add(out=out_sb[:, j, :],
                                             in0=out_sb[:, j, :],
                                             in1=gb_sb[:, 1, :])
                if ch % 2 == 1:
                    nc.scalar.dma_start(out=outv2[ch // 2],
                                        in_=dma_cache["out2"])
                elif ch == nch - 1:
                    nc.scalar.dma_start(out=outv[ch],
                                        in_=dma_cache["out2"][:, 0:4, :])
                if i == GRP - 1 or ch == nch - 1:
                    del live_yn[gi]

            # one-chunk software pipeline: chunk i's front half is emitted
            # before chunk i-1's back half. The Newton batch for a group is
            # emitted right after its last bn_aggr (so DVE runs it before
            # the next chunk's stats), and the group's affines one iteration
            # later (so the ACT queue holds a sigmoid between the Newton
            # chain and the affines that wait on it). pe_ps bufs = GRP + 1.
            enh_live = {}
            for ch in range(nch + GRP + 2):
                if ch < nch:
                    enh_live[ch] = front_half(ch)
                bch = ch - 1
                if 0 <= bch < nch:
                    back_stats(bch, enh_live.pop(bch))
                    if bch % GRP == GRP - 1 or bch == nch - 1:
                        group_newton(bch // GRP, bch % GRP + 1)
                ach = ch - (GRP + 1)
                if 0 <= ach < nch:
                    affine_chunk(ach)

    nc.compile()
    return nc


_NC_CACHE = {}


def _plan(seg, total):
    """Pad each segment to a CHUNK multiple; lay chunks out over cores.

    Returns (nch, chunk_seg [8*nch], node_index [8*nch*CHUNK] int64 with -1
    for padding)."""
    seg = np.asarray(seg)
    counts = np.bincount(seg, minlength=B)[:B]
    chunks_per_seg = (counts + CHUNK - 1) // CHUNK
    total_chunks = int(chunks_per_seg.sum())
    nch = (total_chunks + N_CORES - 1) // N_CORES
    nch = max(nch, 2)
    grid_chunks = N_CORES * nch

    chunk_seg = np.zeros(grid_chunks, np.int64)
    node_index = np.full(grid_chunks * CHUNK, -1, np.int64)
    starts = np.concatenate([[0], np.cumsum(counts)])
    pos = 0
    for s in range(B):
        n = int(counts[s])
        if n == 0:
            continue
        k = int(chunks_per_seg[s])
        chunk_seg[pos:pos + k] = s
        idx = np.arange(starts[s], starts[s] + n)
        node_index[pos * CHUNK: pos * CHUNK + n] = idx
        pos += k
    # remaining chunks (pos..grid) stay segment 0, all padding
    return nch, chunk_seg, node_index


def _make_in_maps(node_feat, text_feat, seg, W1, b1, W2, b2, Wg, bg,
                  ln_gamma, ln_beta, nch, chunk_seg, node_index):
    npc = nch * CHUNK
    node_feat = np.asarray(node_feat, dtype=np.float32)
    # gather into padded layout (zeros in padding), then bf16-transpose
    padded = np.zeros((N_CORES * npc, D), np.float32)
    valid = node_index >= 0
    padded[valid] = node_feat[node_index[valid]]
    nodeT = np.ascontiguousarray(
        padded.T.astype(ml_dtypes.bfloat16))            # [256, 8*npc]
    ohc = (chunk_seg[None, :] == np.arange(B)[:, None]).astype(np.float32)

    textT = np.ascontiguousarray(
        np.asarray(text_feat, np.float32).T.astype(ml_dtypes.bfloat16))
    shared = {
        "textT": textT,
        "w1": np.ascontiguousarray(
            np.asarray(W1, np.float32).astype(ml_dtypes.bfloat16)),
        "b1": np.asarray(b1, np.float32).astype(
            ml_dtypes.bfloat16).reshape(1, HD),
        "w2": np.asarray(W2, np.float32),
        "b2": np.asarray(b2, np.float32).reshape(1, D),
        "wg1": np.ascontiguousarray(
            np.asarray(Wg, np.float32)[:D].astype(ml_dtypes.bfloat16)),
        "wg2": np.ascontiguousarray(np.asarray(Wg, np.float32)[D:]),
        "bg": np.asarray(bg, np.float32).reshape(1, D),
        "gamma": np.asarray(ln_gamma, np.float32).reshape(1, D),
        "beta": np.asarray(ln_beta, np.float32).reshape(1, D),
        "onesd": np.ones((1, B), np.float32),
    }
    in_maps = []
    for c in range(N_CORES):
        m = dict(shared)
        m["nodeT"] = np.ascontiguousarray(nodeT[:, c * npc:(c + 1) * npc])
        m["ohc"] = np.ascontiguousarray(
            ohc[:, c * nch:(c + 1) * nch])
        in_maps.append(m)
    return in_maps


def kernel(node_feat, text_feat, segment_ids, W1, b1, W2, b2, Wg, bg,
           ln_gamma, ln_beta):
    total, d = node_feat.shape
    seg = np.asarray(segment_ids)
    nch, chunk_seg, node_index = _plan(seg, total)

    apply_gb = not (np.all(np.asarray(ln_gamma) == 1.0)
                    and np.all(np.asarray(ln_beta) == 0.0))

    key = (nch, apply_gb)
    if key not in _NC_CACHE:
        _NC_CACHE[key] = _build(nch, apply_gb)
    nc = _NC_CACHE[key]

    in_maps = _make_in_maps(node_feat, text_feat, seg, W1, b1, W2, b2, Wg,
                            bg, ln_gamma, ln_beta, nch, chunk_seg, node_index)

    res = run_bass_kernel_spmd(nc, in_maps, core_ids=list(range(N_CORES)))
    out_pad = np.concatenate(
        [res.results[c]["out"] for c in range(N_CORES)], axis=0)
    valid = node_index >= 0
    out = np.empty((total, D), np.float32)
    out[node_index[valid]] = out_pad[valid]
    return out


def bench_device(inputs, iters=12):
    """Steady-state wall time per on-device execution (8 cores, inputs
    device-resident, donated outputs chained call-to-call). Includes PJRT
    dispatch overhead; see run_traced for the profiled HW time."""
    import time

    import jax
    from jax.experimental.shard_map import shard_map
    from jax.sharding import Mesh, PartitionSpec

    import concourse.bass2jax as b2j
    import concourse.mybir as mb

    seg = np.asarray(inputs["segment_ids"])
    total = np.asarray(inputs["node_feat"]).shape[0]
    nch, chunk_seg, node_index = _plan(seg, total)
    key = (nch, False)
    if key not in _NC_CACHE:
        _NC_CACHE[key] = _build(nch, False)
    nc = _NC_CACHE[key]
    in_maps = _make_in_maps(
        inputs["node_feat"], inputs["text_feat"], seg, inputs["W1"],
        inputs["b1"], inputs["W2"], inputs["b2"], inputs["Wg"], inputs["bg"],
        inputs["ln_gamma"], inputs["ln_beta"], nch, chunk_seg, node_index)

    b2j.install_neuronx_cc_hook()
    partition_name = (nc.partition_id_tensor.name
                      if nc.partition_id_tensor else None)
    in_names, out_names, out_avals, zero_outs = [], [], [], []
    for alloc in nc.m.functions[0].allocations:
        if not isinstance(alloc, mb.MemoryLocationSet):
            continue
        name = alloc.memorylocations[0].name
        if alloc.kind == "ExternalInput":
            if name != partition_name:
                in_names.append(name)
        elif alloc.kind == "ExternalOutput":
            out_names.append(name)
            shape = tuple(alloc.tensor_shape)
            dtype = mb.dt.np(alloc.dtype)
            out_avals.append(jax.core.ShapedArray(shape, dtype))
            zero_outs.append(np.zeros(shape, dtype))
    n_params = len(in_names)
    n_outs = len(out_avals)
    in_names_all = list(in_names) + out_names
    if partition_name is not None:
        in_names_all.append(partition_name)
    donate = tuple(range(n_params, n_params + n_outs))

    def _body(*args):
        operands = list(args)
        if partition_name is not None:
            operands.append(b2j.partition_id_tensor())
        outs = b2j._bass_exec_p.bind(
            *operands, out_avals=tuple(out_avals), in_names=tuple(in_names_all),
            out_names=tuple(out_names), lowering_input_output_aliases=(),
            sim_require_finite=True, sim_require_nnan=True, nc=nc)
        return tuple(outs)

    devices = jax.devices()[:N_CORES]
    mesh = Mesh(np.asarray(devices), ("core",))
    sharded = jax.jit(
        shard_map(_body, mesh=mesh,
                  in_specs=(PartitionSpec("core"),) * (n_params + n_outs),
                  out_specs=(PartitionSpec("core"),) * n_outs,
                  check_rep=False),
        donate_argnums=donate, keep_unused=True)
    concat_in = [
        np.concatenate([np.asarray(in_maps[c][nm]) for c in range(N_CORES)],
                       axis=0)
        for nm in in_names]
    sh = jax.sharding.NamedSharding(mesh, PartitionSpec("core"))
    in_dev = [jax.device_put(a, sh) for a in concat_in]
    zs = [jax.device_put(
        np.zeros((N_CORES * z.shape[0], *z.shape[1:]), z.dtype), sh)
        for z in zero_outs]
    jax.block_until_ready(in_dev)
    jax.block_until_ready(zs)
    outs = sharded(*in_dev, *zs)
    jax.block_until_ready(outs)          # warm-up / compile
    times = []
    for it in range(iters):
        t0 = time.perf_counter()
        nxt = sharded(*in_dev, *outs)
        jax.block_until_ready(nxt)
        times.append(time.perf_counter() - t0)
        outs = nxt
    times.sort()
    return times[len(times) // 2], times


def run_traced(inputs, trace_cores=None):
    """Re-run with NTFF tracing; returns max-core exec time in ns (or None)."""
    global _LAST_TRACE
    seg = np.asarray(inputs["segment_ids"])
    total = np.asarray(inputs["node_feat"]).shape[0]
    nch, chunk_seg, node_index = _plan(seg, total)
    apply_gb = not (np.all(np.asarray(inputs["ln_gamma"]) == 1.0)
                    and np.all(np.asarray(inputs["ln_beta"]) == 0.0))
    key = (nch, apply_gb)
    if key not in _NC_CACHE:
        _NC_CACHE[key] = _build(nch, apply_gb)
    nc = _NC_CACHE[key]
    in_maps = _make_in_maps(
        inputs["node_feat"], inputs["text_feat"], seg, inputs["W1"],
        inputs["b1"], inputs["W2"], inputs["b2"], inputs["Wg"], inputs["bg"],
        inputs["ln_gamma"], inputs["ln_beta"], nch, chunk_seg, node_index)
    res = run_bass_kernel_spmd(nc, in_maps, core_ids=list(range(N_CORES)),
                               trace=True, trace_cores=trace_cores)
    _LAST_TRACE = res
    return res.exec_time_ns
